# revision 2
# baseline (speedup 1.0000x reference)
"""HOPE block kernel for 8 Trainium2 NeuronCores.

Sharding: attention is head-parallel (8 heads -> 8 cores, each core runs the
full-sequence linear-attention scan for its head locally), everything
token-parallel elsewhere (layernorms, output projection, CMS MLPs on a
1024-token shard per core).  Cross-core movement: one AllGather of the
ln1-normalized activations (transposed, bf16) + one AllToAll of the
per-head attention outputs.

M = cumsum_t(mean_b V K^T) readout is computed with a chunked scan:
  Y^T_chunk = G^T Q^T + V^T (mask o K Q^T),  G += sum_b V_b^T K_b
with the 1/B scale and the ln scale/bias folded into the projection weights
host-side.
"""

import numpy as np
import ml_dtypes

import concourse.bass as bass
import concourse.bacc as bacc
import concourse.mybir as mybir
import concourse.tile as tile
from concourse.bass_utils import run_bass_kernel_spmd
from concourse.masks import make_identity

N_CORES = 8
B, S, DIM = 4, 2048, 512
H, D = 8, 64
HID = 4 * DIM
NLVL = 3
EPS = 1e-5
TOK = B * S              # 8192 flat tokens
TSH = TOK // N_CORES     # 1024 tokens per shard
NT = TSH // 128          # 8 token tiles per shard
NCH = S // 128           # 16 sequence chunks per batch
FP32 = mybir.dt.float32
BF16 = mybir.dt.bfloat16
AX = mybir.AxisListType.X
ALU = mybir.AluOpType
ACTF = mybir.ActivationFunctionType


def _ln_normalize(nc, pool, xt, out_bf, sq_scratch, eps_tile):
    """out_bf = (xt - mean(xt)) * rsqrt(var(xt)+EPS), per 128-token tile."""
    ssum = pool.tile([128, 1], FP32, tag="ln_s")
    sumsq = pool.tile([128, 1], FP32, tag="ln_q")
    nc.vector.tensor_reduce(ssum[:], xt[:], AX, ALU.add)
    nc.scalar.activation(sq_scratch[:], xt[:], ACTF.Square, accum_out=sumsq[:])
    negmu = pool.tile([128, 1], FP32, tag="ln_m")
    nc.vector.tensor_scalar_mul(negmu[:], ssum[:], -1.0 / DIM)
    e2 = pool.tile([128, 1], FP32, tag="ln_e")
    nc.vector.tensor_scalar_mul(e2[:], sumsq[:], 1.0 / DIM)
    mu2 = pool.tile([128, 1], FP32, tag="ln_2")
    nc.vector.tensor_tensor(mu2[:], negmu[:], negmu[:], ALU.mult)
    var = pool.tile([128, 1], FP32, tag="ln_v")
    nc.vector.tensor_tensor(var[:], e2[:], mu2[:], ALU.subtract)
    std = pool.tile([128, 1], FP32, tag="ln_d")
    nc.scalar.activation(std[:], var[:], ACTF.Sqrt, bias=eps_tile[:])
    rs = pool.tile([128, 1], FP32, tag="ln_r")
    nc.vector.reciprocal(rs[:], std[:])
    nc.vector.tensor_scalar(
        out=out_bf[:], in0=xt[:], scalar1=negmu[:], scalar2=rs[:],
        op0=ALU.add, op1=ALU.mult,
    )


def build_kernel():
    nc = bacc.Bacc(num_devices=N_CORES)

    x_sh = nc.dram_tensor("x_shard", [TSH, DIM], FP32, kind="ExternalInput")
    qk_wT = nc.dram_tensor("qk_wT", [DIM, 128], BF16, kind="ExternalInput")
    v_wT = nc.dram_tensor("v_wT", [DIM, D], BF16, kind="ExternalInput")
    qkv_b = nc.dram_tensor("qkv_b", [3, D, 1], FP32, kind="ExternalInput")
    wo_T = nc.dram_tensor("wo_T", [DIM, DIM], BF16, kind="ExternalInput")
    w1 = nc.dram_tensor("w1", [NLVL, DIM, HID], BF16, kind="ExternalInput")
    w2 = nc.dram_tensor("w2", [NLVL, HID, DIM], BF16, kind="ExternalInput")
    b1 = nc.dram_tensor("b1", [NLVL, 128, HID // 128], FP32, kind="ExternalInput")
    b2a = nc.dram_tensor("b2a", [2, 128, DIM // 128], FP32, kind="ExternalInput")
    b2last = nc.dram_tensor("b2last", [128, DIM], FP32, kind="ExternalInput")
    out_sh = nc.dram_tensor("out_shard", [TSH, DIM], FP32, kind="ExternalOutput")

    with tile.TileContext(nc) as tc:
        with tc.tile_pool(name="dram", bufs=1, space="DRAM") as dram, \
             tc.tile_pool(name="const", bufs=1) as cpool, \
             tc.tile_pool(name="lns", bufs=4) as lnp, \
             tc.tile_pool(name="xp", bufs=1) as xpool:

            ag_in = dram.tile([DIM, TSH], BF16)
            ag_out = dram.tile([N_CORES * DIM, TSH], BF16)
            a2a_in = dram.tile([N_CORES * D, TSH], BF16)
            a2a_out = dram.tile([N_CORES * D, TSH], BF16)

            identity = cpool.tile([128, 128], BF16)
            make_identity(nc, identity[:])
            identity64 = cpool.tile([D, D], BF16)
            make_identity(nc, identity64[:])
            # keep-mask: mask[t, s] = 1.0 if t <= s else 0.0
            mask = cpool.tile([128, 128], FP32)
            nc.gpsimd.memset(mask[:], 1.0)
            nc.gpsimd.affine_select(
                out=mask[:], in_=mask[:], compare_op=ALU.is_ge, fill=0.0,
                base=0, pattern=[[1, 128]], channel_multiplier=-1,
            )

            qkw_sb = cpool.tile([128, 4, 128], BF16)
            nc.sync.dma_start(qkw_sb[:], qk_wT[:].rearrange("(a p) m -> p a m", p=128))
            vw_sb = cpool.tile([128, 4, D], BF16)
            nc.sync.dma_start(vw_sb[:], v_wT[:].rearrange("(a p) m -> p a m", p=128))
            qb_sb = cpool.tile([D, 3], FP32)
            nc.sync.dma_start(qb_sb[:], qkv_b[:].rearrange("c p one -> p (c one)"))
            woT_sb = cpool.tile([128, 4, DIM], BF16)
            nc.sync.dma_start(woT_sb[:], wo_T[:].rearrange("(a p) m -> p a m", p=128))
            b2l_sb = cpool.tile([128, DIM], FP32)
            nc.sync.dma_start(b2l_sb[:], b2last[:])
            eps_sb = cpool.tile([128, 1], FP32)
            nc.vector.memset(eps_sb[:], EPS)

            x_sb = xpool.tile([128, NT, DIM], FP32)
            nc.sync.dma_start(x_sb[:], x_sh[:].rearrange("(t p) d -> p t d", p=128))

            # ---- stage 1: ln1 on own token shard, transpose, AllGather ----
            with tc.tile_pool(name="hT", bufs=1) as hTp, \
                 tc.tile_pool(name="s1w", bufs=3) as s1w, \
                 tc.tile_pool(name="s1p", bufs=2, space="PSUM") as s1p:
                hT_sb = hTp.tile([128, 4, TSH], BF16)
                for t in range(NT):
                    hn = s1w.tile([128, DIM], BF16, tag="hn")
                    sq = s1w.tile([128, DIM], BF16, tag="sq")
                    _ln_normalize(nc, lnp, x_sb[:, t], hn, sq, eps_sb)
                    for a in range(4):
                        ps = s1p.tile([128, 128], BF16)
                        nc.tensor.transpose(ps[:], hn[:, a * 128:(a + 1) * 128], identity[:])
                        nc.vector.tensor_copy(hT_sb[:, a, t * 128:(t + 1) * 128], ps[:])
                nc.sync.dma_start(ag_in[:].rearrange("(a p) n -> p a n", p=128), hT_sb[:])

            nc.gpsimd.collective_compute(
                "AllGather", ALU.bypass,
                replica_groups=[list(range(N_CORES))],
                ins=[ag_in.opt()], outs=[ag_out.opt()],
            )
            agv = ag_out[:].rearrange("(s a p) n -> s a p n", s=N_CORES, a=4)

            # ---- stage 2+3: per-head QKV projections + chunked scan ----
            with tc.tile_pool(name="qkv", bufs=1) as qkvp, \
                 tc.tile_pool(name="s2w", bufs=4) as s2w:
                s2ctx = tc.tile_pool(name="s2p", bufs=2, space="PSUM")
                s2p = s2ctx.__enter__()
                s2vctx = tc.tile_pool(name="s2pv", bufs=2, space="PSUM")
                s2pv = s2vctx.__enter__()
                s2tctx = tc.tile_pool(name="s2pt", bufs=2, space="PSUM")
                s2pt = s2tctx.__enter__()
                QT = qkvp.tile([D, TOK], BF16)
                KT = qkvp.tile([D, TOK], BF16)
                VT = qkvp.tile([D, TOK], BF16)
                K_td = qkvp.tile([128, 64 * D], BF16)
                V_td = qkvp.tile([128, 64 * D], BF16)

                for blk in range(16):          # 512-token blocks
                    s, h2 = blk // 2, blk % 2
                    col = slice(blk * 512, (blk + 1) * 512)
                    rhs = []
                    for a in range(4):
                        r = s2w.tile([128, 512], BF16, tag="rhs")
                        nc.sync.dma_start(r[:], agv[s, a, :, h2 * 512:(h2 + 1) * 512])
                        rhs.append(r)
                    pqk = s2p.tile([128, 512], FP32)
                    for a in range(4):
                        nc.tensor.matmul(pqk[:], qkw_sb[:, a], rhs[a][:],
                                         start=(a == 0), stop=(a == 3))
                    pv = s2pv.tile([D, 512], FP32)
                    for a in range(4):
                        nc.tensor.matmul(pv[:], vw_sb[:, a], rhs[a][:],
                                         start=(a == 0), stop=(a == 3))
                    nc.scalar.activation(QT[:, col], pqk[0:D, :], ACTF.Identity,
                                         bias=qb_sb[:, 0:1])
                    nc.scalar.activation(KT[:, col], pqk[D:128, :], ACTF.Identity,
                                         bias=qb_sb[:, 1:2])
                    nc.scalar.activation(VT[:, col], pv[:], ACTF.Identity,
                                         bias=qb_sb[:, 2:3])
                    for u in range(4):         # transpose K,V 128-tok subtiles
                        tt = blk * 4 + u
                        csub = slice(tt * 128, (tt + 1) * 128)
                        pk = s2pt.tile([128, D], BF16, tag="pk")
                        nc.tensor.transpose(pk[:], KT[:, csub], identity64[:])
                        nc.vector.tensor_copy(K_td[:, tt * D:(tt + 1) * D], pk[:])
                        pvv = s2pt.tile([128, D], BF16, tag="pv")
                        nc.tensor.transpose(pvv[:], VT[:, csub], identity64[:])
                        nc.vector.tensor_copy(V_td[:, tt * D:(tt + 1) * D], pvv[:])

                s2tctx.__exit__(None, None, None)
                s2vctx.__exit__(None, None, None)
                s2ctx.__exit__(None, None, None)
                # ---- chunked scan ----
                G32 = qkvp.tile([D, D], FP32)
                G16 = qkvp.tile([D, D], BF16)
                nc.vector.memset(G32[:], 0.0)
                nc.vector.memset(G16[:], 0.0)

                with tc.tile_pool(name="tm", bufs=18) as tmp_pool, \
                     tc.tile_pool(name="yev", bufs=4) as yev, \
                     tc.tile_pool(name="pt3", bufs=3, space="PSUM") as pt3, \
                     tc.tile_pool(name="py3", bufs=2, space="PSUM") as py3, \
                     tc.tile_pool(name="pg3", bufs=1, space="PSUM") as pg3:
                    for sc in range(NCH):
                        ttb = [bb * NCH + sc for bb in range(B)]
                        Tm = {}
                        for bp in range(B):
                            kcol = slice(ttb[bp] * 128, ttb[bp] * 128 + 128)
                            for b in range(B):
                                qcol = slice(ttb[b] * 128, ttb[b] * 128 + 128)
                                pt = pt3.tile([128, 128], FP32)
                                nc.tensor.matmul(pt[:], KT[:, kcol], QT[:, qcol])
                                tm = tmp_pool.tile([128, 128], BF16)
                                nc.vector.tensor_tensor(tm[:], pt[:], mask[:], ALU.mult)
                                Tm[(bp, b)] = tm
                        for b in range(B):
                            qcol = slice(ttb[b] * 128, ttb[b] * 128 + 128)
                            py = py3.tile([D, 128], FP32)
                            nc.tensor.matmul(py[:], G16[:], QT[:, qcol],
                                             start=True, stop=False)
                            for bp in range(B):
                                vcol = slice(ttb[bp] * D, (ttb[bp] + 1) * D)
                                nc.tensor.matmul(py[:], V_td[:, vcol], Tm[(bp, b)][:],
                                                 start=False, stop=(bp == B - 1))
                            ye = yev.tile([D, 128], BF16)
                            nc.scalar.activation(ye[:], py[:], ACTF.Copy)
                            flat = b * S + sc * 128
                            j, off = flat // TSH, flat % TSH
                            nc.sync.dma_start(
                                a2a_in[j * D:(j + 1) * D, off:off + 128], ye[:])
                        pg = pg3.tile([D, D], FP32)
                        for b in range(B):
                            vcol = slice(ttb[b] * D, (ttb[b] + 1) * D)
                            nc.tensor.matmul(pg[:], K_td[:, vcol], V_td[:, vcol],
                                             start=(b == 0), stop=(b == B - 1))
                        nc.vector.tensor_tensor(G32[:], G32[:], pg[:], ALU.add)
                        nc.vector.tensor_copy(G16[:], G32[:])

            nc.gpsimd.collective_compute(
                "AllToAll", ALU.bypass,
                replica_groups=[list(range(N_CORES))],
                ins=[a2a_in.opt()], outs=[a2a_out.opt()],
            )

            # ---- stage 4: Wo + residual + ln2 + transpose ----
            h2nT_p = tc.tile_pool(name="h2nT", bufs=1)
            h2_p = tc.tile_pool(name="h2", bufs=1)
            h2nT = h2nT_p.__enter__().tile([128, 4, TSH], BF16)
            h2_sb = h2_p.__enter__().tile([128, NT, DIM], FP32)
            with tc.tile_pool(name="s4w", bufs=4) as s4w, \
                 tc.tile_pool(name="s4p", bufs=2, space="PSUM") as s4p, \
                 tc.tile_pool(name="s4pt", bufs=2, space="PSUM") as s4pt:
                for t in range(NT):
                    po = s4p.tile([128, DIM], FP32)
                    for dk in range(4):
                        lh = s4w.tile([128, 128], BF16, tag="yT")
                        nc.sync.dma_start(
                            lh[:], a2a_out[dk * 128:(dk + 1) * 128, t * 128:(t + 1) * 128])
                        nc.tensor.matmul(po[:], lh[:], woT_sb[:, dk],
                                         start=(dk == 0), stop=(dk == 3))
                    nc.vector.tensor_tensor(h2_sb[:, t], po[:], x_sb[:, t], ALU.add)
                    hn = s4w.tile([128, DIM], BF16, tag="hn2")
                    sq = s4w.tile([128, DIM], BF16, tag="sq2")
                    _ln_normalize(nc, lnp, h2_sb[:, t], hn, sq, eps_sb)
                    for a in range(4):
                        ps = s4pt.tile([128, 128], BF16)
                        nc.tensor.transpose(ps[:], hn[:, a * 128:(a + 1) * 128], identity[:])
                        nc.vector.tensor_copy(h2nT[:, a, t * 128:(t + 1) * 128], ps[:])

            # ---- stage 5: CMS chain ----
            with tc.tile_pool(name="g", bufs=1) as gp, \
                 tc.tile_pool(name="wts", bufs=2) as wts, \
                 tc.tile_pool(name="bts", bufs=2) as bts, \
                 tc.tile_pool(name="s5o", bufs=3) as s5o, \
                 tc.tile_pool(name="s5p", bufs=4, space="PSUM") as s5p:
                g_sb = gp.tile([128, 16, TSH], BF16)
                cur = h2nT
                for lvl in range(NLVL):
                    w1_sb = wts.tile([128, 4, 16, 128], BF16, tag="w")
                    nc.sync.dma_start(
                        w1_sb[:],
                        w1[lvl].rearrange("(a p) (ht q) -> p a ht q", p=128, q=128))
                    b1_sb = bts.tile([128, HID // 128], FP32, tag="b1")
                    nc.sync.dma_start(b1_sb[:], b1[lvl])
                    for ht in range(16):
                        for nh in range(2):
                            colw = slice(nh * 512, nh * 512 + 512)
                            ps = s5p.tile([128, 512], FP32)
                            for a in range(4):
                                nc.tensor.matmul(ps[:], w1_sb[:, a, ht],
                                                 cur[:, a, colw],
                                                 start=(a == 0), stop=(a == 3))
                            nc.scalar.activation(
                                g_sb[:, ht, colw], ps[:], ACTF.Gelu_apprx_tanh,
                                bias=b1_sb[:, ht:ht + 1])
                    w2_sb = wts.tile([128, 16, 4, 128], BF16, tag="w")
                    nc.sync.dma_start(
                        w2_sb[:],
                        w2[lvl].rearrange("(ht p) (a q) -> p ht a q", p=128, q=128))
                    if lvl < 2:
                        b2_sb = bts.tile([128, 4], FP32, tag="b2")
                        nc.sync.dma_start(b2_sb[:], b2a[lvl])
                        nxt = s5o.tile([128, 4, TSH], BF16, tag="nxt")
                        for a in range(4):
                            for nh in range(2):
                                colw = slice(nh * 512, nh * 512 + 512)
                                ps = s5p.tile([128, 512], FP32)
                                for ht in range(16):
                                    nc.tensor.matmul(ps[:], w2_sb[:, ht, a],
                                                     g_sb[:, ht, colw],
                                                     start=(ht == 0), stop=(ht == 15))
                                nc.scalar.activation(
                                    nxt[:, a, colw], ps[:], ACTF.Identity,
                                    bias=b2_sb[:, a:a + 1])
                        cur = nxt
                    else:
                        # last level: emit [tok, f], add b2 + residual, write out
                        w2r_sb = wts.tile([128, 16, 512], BF16, tag="w2r")
                        nc.sync.dma_start(
                            w2r_sb[:],
                            w2[2].rearrange("(ht p) m -> p ht m", p=128))
                        for t in range(NT):
                            ps = s5p.tile([128, 512], FP32)
                            for ht in range(16):
                                nc.tensor.matmul(
                                    ps[:], g_sb[:, ht, t * 128:(t + 1) * 128],
                                    w2r_sb[:, ht],
                                    start=(ht == 0), stop=(ht == 15))
                            tmp = s5o.tile([128, DIM], FP32, tag="fin")
                            nc.vector.tensor_tensor(tmp[:], ps[:], b2l_sb[:], ALU.add)
                            nc.vector.tensor_tensor(tmp[:], tmp[:], h2_sb[:, t], ALU.add)
                            nc.sync.dma_start(
                                out_sh[:].rearrange("(t p) d -> p t d", p=128)[:, t],
                                tmp[:])
            h2_p.__exit__(None, None, None)
            h2nT_p.__exit__(None, None, None)

    nc.finalize()
    return nc


_NC_CACHE = {}


def _get_nc():
    if "nc" not in _NC_CACHE:
        _NC_CACHE["nc"] = build_kernel()
    return _NC_CACHE["nc"]


def kernel(x, Wq, Wk, Wv, Wo, ln1_w, ln1_b, ln2_w, ln2_b,
           cms_W1, cms_b1, cms_W2, cms_b2):
    bf = ml_dtypes.bfloat16
    f32 = np.float32
    x = np.asarray(x, f32)
    ln1_w = np.asarray(ln1_w, f32); ln1_b = np.asarray(ln1_b, f32)
    ln2_w = np.asarray(ln2_w, f32); ln2_b = np.asarray(ln2_b, f32)

    # fold ln1 scale into Wq/Wk/Wv columns, ln1 bias into additive biases
    Wq = np.asarray(Wq, f32); Wk = np.asarray(Wk, f32); Wv = np.asarray(Wv, f32)
    Wo = np.asarray(Wo, f32)
    Wqs = Wq * ln1_w[None, :]; Wks = Wk * ln1_w[None, :]
    Wvs = (Wv * ln1_w[None, :]) / B
    bq = Wq @ ln1_b; bk = Wk @ ln1_b; bv = (Wv @ ln1_b) / B

    W1 = np.asarray(cms_W1, f32).copy(); b1v = np.asarray(cms_b1, f32).copy()
    W2 = np.asarray(cms_W2, f32); b2v = np.asarray(cms_b2, f32)
    b1v[0] = b1v[0] + ln2_b @ W1[0]
    W1[0] = W1[0] * ln2_w[:, None]

    xf = x.reshape(TOK, DIM)
    b1r = np.ascontiguousarray(
        b1v.reshape(NLVL, HID // 128, 128).transpose(0, 2, 1))
    b2ar = np.ascontiguousarray(
        b2v[:2].reshape(2, DIM // 128, 128).transpose(0, 2, 1))
    b2last = np.broadcast_to(b2v[2], (128, DIM)).copy()

    in_maps = []
    for c in range(N_CORES):
        hs = slice(c * D, (c + 1) * D)
        qk_wT = np.concatenate([Wqs[hs].T, Wks[hs].T], axis=1)  # [512, 128]
        in_maps.append({
            "x_shard": np.ascontiguousarray(xf[c * TSH:(c + 1) * TSH]),
            "qk_wT": qk_wT.astype(bf),
            "v_wT": np.ascontiguousarray(Wvs[hs].T).astype(bf),
            "qkv_b": np.stack([bq[hs], bk[hs], bv[hs]])[:, :, None].astype(f32),
            "wo_T": np.ascontiguousarray(Wo.T).astype(bf),
            "w1": W1.astype(bf),
            "w2": W2.astype(bf),
            "b1": b1r.astype(f32),
            "b2a": b2ar.astype(f32),
            "b2last": b2last.astype(f32),
        })

    nc = _get_nc()
    res = run_bass_kernel_spmd(nc, in_maps, core_ids=list(range(N_CORES)))
    _NC_CACHE["last_result"] = res
    out = np.concatenate([res.results[c]["out_shard"] for c in range(N_CORES)], axis=0)
    return out.reshape(B, S, DIM).astype(np.float32)



# revision 11
# speedup vs baseline: 1.4795x; 1.4795x over previous
"""HOPE block kernel for 8 Trainium2 NeuronCores.

Sequence-parallel sharding: core c owns timesteps [256c, 256(c+1)) of all 4
batches (1024 tokens) and computes ALL 8 heads locally.  The linear-attention
memory M = cumsum_t(mean_b v k^T) is split into a local (within-shard) masked
scan plus a cross-core prefix: each core AllGathers its per-shard memory sum
G_c (8 heads x 64x64, 64KB bf16) and folds sum_{c'<c} G_c' in with a
0/1-mask matmul.  Everything else (LN1/QKV/scan/Wo/LN2/CMS) is local.

ln scales/biases and the 1/B factor are folded into the projection weights
host-side; all weights are pre-arranged host-side so every DMA is a plain
[128, contiguous] transfer.
"""

import numpy as np
import ml_dtypes

import concourse.bass as bass
import concourse.bacc as bacc
import concourse.mybir as mybir
import concourse.tile as tile
from concourse.bass_utils import run_bass_kernel_spmd
from concourse.masks import make_identity

N_CORES = 8
B, S, DIM = 4, 2048, 512
H, D = 8, 64
HID = 4 * DIM
NLVL = 3
EPS = 1e-5
SSH = S // N_CORES       # 256 timesteps per core
TSH = B * SSH            # 1024 tokens per core
NT = TSH // 128          # 8 token tiles (tile t = chunk(t//4), batch(t%4))
NCH = SSH // 128         # 2 chunks of 128 steps
FP32 = mybir.dt.float32
BF16 = mybir.dt.bfloat16
AX = mybir.AxisListType.X
ALU = mybir.AluOpType
ACTF = mybir.ActivationFunctionType


def _ln_normalize(nc, pool, xt, out_bf, sq_scratch, eps_tile):
    """out_bf = (xt - mean(xt)) * rsqrt(var(xt)+EPS), per 128-token tile."""
    ssum = pool.tile([128, 1], FP32, tag="ln_s")
    sumsq = pool.tile([128, 1], FP32, tag="ln_q")
    nc.vector.tensor_reduce(ssum[:], xt[:], AX, ALU.add)
    nc.scalar.activation(sq_scratch[:], xt[:], ACTF.Square, accum_out=sumsq[:])
    negmu = pool.tile([128, 1], FP32, tag="ln_m")
    nc.vector.tensor_scalar_mul(negmu[:], ssum[:], -1.0 / DIM)
    e2 = pool.tile([128, 1], FP32, tag="ln_e")
    nc.vector.tensor_scalar_mul(e2[:], sumsq[:], 1.0 / DIM)
    mu2 = pool.tile([128, 1], FP32, tag="ln_2")
    nc.vector.tensor_tensor(mu2[:], negmu[:], negmu[:], ALU.mult)
    var = pool.tile([128, 1], FP32, tag="ln_v")
    nc.vector.tensor_tensor(var[:], e2[:], mu2[:], ALU.subtract)
    std = pool.tile([128, 1], FP32, tag="ln_d")
    nc.scalar.activation(std[:], var[:], ACTF.Sqrt, bias=eps_tile[:])
    rs = pool.tile([128, 1], FP32, tag="ln_r")
    nc.vector.reciprocal(rs[:], std[:])
    nc.vector.tensor_scalar(
        out=out_bf[:], in0=xt[:], scalar1=negmu[:], scalar2=rs[:],
        op0=ALU.add, op1=ALU.mult,
    )


def build_kernel():
    nc = bacc.Bacc(num_devices=N_CORES)

    x_t = nc.dram_tensor("x_t", [128, NT, DIM], FP32, kind="ExternalInput")
    qkw = nc.dram_tensor("qkw", [128, 4, H, 128], BF16, kind="ExternalInput")
    qk_b = nc.dram_tensor("qk_b", [128, H], FP32, kind="ExternalInput")
    wkT = nc.dram_tensor("wkT", [128, 4, DIM], BF16, kind="ExternalInput")
    wvT = nc.dram_tensor("wvT", [128, 4, DIM], BF16, kind="ExternalInput")
    bkv = nc.dram_tensor("bkv", [128, 2 * DIM], FP32, kind="ExternalInput")
    woT = nc.dram_tensor("woT", [128, 4, DIM], BF16, kind="ExternalInput")
    sel = nc.dram_tensor("sel", [128, 4, D], BF16, kind="ExternalInput")
    w1 = nc.dram_tensor("w1", [NLVL, 128, 4 * 16 * 128], BF16, kind="ExternalInput")
    w2 = nc.dram_tensor("w2", [NLVL, 128, 16 * 4 * 128], BF16, kind="ExternalInput")
    b1 = nc.dram_tensor("b1", [NLVL, 128, HID // 128], FP32, kind="ExternalInput")
    b2a = nc.dram_tensor("b2a", [2, 128, DIM // 128], FP32, kind="ExternalInput")
    b2last = nc.dram_tensor("b2last", [128, DIM], FP32, kind="ExternalInput")
    out_t = nc.dram_tensor("out_t", [128, NT, DIM], FP32, kind="ExternalOutput")

    with tile.TileContext(nc) as tc:
        with tc.tile_pool(name="dram", bufs=1, space="DRAM") as dram, \
             tc.tile_pool(name="const", bufs=1) as cpool, \
             tc.tile_pool(name="lns", bufs=4) as lnp, \
             tc.tile_pool(name="xp", bufs=1) as xpool, \
             tc.tile_pool(name="cmsw", bufs=2) as wts:

            ag_in = dram.tile([D, DIM], BF16)
            ag_out = dram.tile([N_CORES * D, DIM], BF16)

            identity = cpool.tile([128, 128], BF16)
            make_identity(nc, identity[:])
            # keep-mask tiled over 4 batch column blocks: mask[t, s%128]=1 if t<=s
            mask = cpool.tile([128, 512], FP32)
            nc.gpsimd.memset(mask[:], 1.0)
            for bb in range(4):
                nc.gpsimd.affine_select(
                    out=mask[:, bb * 128:(bb + 1) * 128],
                    in_=mask[:, bb * 128:(bb + 1) * 128],
                    compare_op=ALU.is_ge, fill=0.0,
                    base=0, pattern=[[1, 128]], channel_multiplier=-1,
                )

            qkw_sb = cpool.tile([128, 4, H, 128], BF16)
            nc.sync.dma_start(qkw_sb[:], qkw[:])
            qkb_sb = cpool.tile([128, H], FP32)
            nc.sync.dma_start(qkb_sb[:], qk_b[:])
            wkT_sb = cpool.tile([128, 4, DIM], BF16)
            nc.sync.dma_start(wkT_sb[:], wkT[:])
            wvT_sb = cpool.tile([128, 4, DIM], BF16)
            nc.sync.dma_start(wvT_sb[:], wvT[:])
            bkv_sb = cpool.tile([128, 2 * DIM], FP32)
            nc.sync.dma_start(bkv_sb[:], bkv[:])
            woT_sb = cpool.tile([128, 4, DIM], BF16)
            nc.sync.dma_start(woT_sb[:], woT[:])
            sel_sb = cpool.tile([128, 4, D], BF16)
            nc.sync.dma_start(sel_sb[:], sel[:])
            b2l_sb = cpool.tile([128, DIM], FP32)
            nc.sync.dma_start(b2l_sb[:], b2last[:])
            eps_sb = cpool.tile([128, 1], FP32)
            nc.vector.memset(eps_sb[:], EPS)

            # prefetch CMS level-0 weights early (stream the rest later)
            w1_sb0 = wts.tile([128, 4, 16, 128], BF16, tag="w1")
            nc.sync.dma_start(w1_sb0[:], w1[0].rearrange("p (a h q) -> p a h q", a=4, h=16))
            w2_sb0 = wts.tile([128, 16, 4, 128], BF16, tag="w2")
            nc.sync.dma_start(w2_sb0[:], w2[0].rearrange("p (h a q) -> p h a q", h=16, a=4))

            x_sb = xpool.tile([128, NT, DIM], FP32)
            nc.sync.dma_start(x_sb[:], x_t[:])

            # long-lived attention tiles
            yT_p = tc.tile_pool(name="yTp", bufs=1)
            yT = yT_p.__enter__().tile([128, 4, TSH], BF16)

            with tc.tile_pool(name="hT", bufs=1) as hTp, \
                 tc.tile_pool(name="kv", bufs=1) as kvp, \
                 tc.tile_pool(name="s1w", bufs=3) as s1w:
                hT = hTp.tile([128, 4, TSH], BF16)
                K_td = kvp.tile([128, NT, DIM], BF16)
                V_td = kvp.tile([128, NT, DIM], BF16)
                GcA_sb = kvp.tile([D, DIM], FP32)
                GcB_sb = kvp.tile([D, DIM], FP32)
                Gc_bf = kvp.tile([D, DIM], BF16)

                # ---- stage 1: ln1 per token tile + transpose to [dim, tok] ----
                with tc.tile_pool(name="s1p", bufs=2, space="PSUM") as s1p:
                    for t in range(NT):
                        hn = s1w.tile([128, DIM], BF16, tag="hn")
                        sq = s1w.tile([128, DIM], BF16, tag="sq")
                        _ln_normalize(nc, lnp, x_sb[:, t], hn, sq, eps_sb)
                        for a in range(4):
                            ps = s1p.tile([128, 128], BF16)
                            nc.tensor.transpose(ps[:], hn[:, a * 128:(a + 1) * 128],
                                                identity[:])
                            nc.vector.tensor_copy(hT[:, a, t * 128:(t + 1) * 128], ps[:])

                # ---- stage 2a: token-side K/V projections + local memory sums ----
                with tc.tile_pool(name="s2aw", bufs=2, space="PSUM") as s2ap, \
                     tc.tile_pool(name="s2g", bufs=1, space="PSUM") as s2gp:
                    pgA = s2gp.tile([D, DIM], FP32)
                    pgB = s2gp.tile([D, DIM], FP32)
                    for t in range(NT):
                        tcol = slice(t * 128, (t + 1) * 128)
                        psK = s2ap.tile([128, DIM], FP32, tag="psK")
                        psV = s2ap.tile([128, DIM], FP32, tag="psV")
                        for a in range(4):
                            nc.tensor.matmul(psK[:], hT[:, a, tcol], wkT_sb[:, a],
                                             start=(a == 0), stop=(a == 3))
                        for a in range(4):
                            nc.tensor.matmul(psV[:], hT[:, a, tcol], wvT_sb[:, a],
                                             start=(a == 0), stop=(a == 3))
                        nc.vector.tensor_tensor(K_td[:, t], psK[:], bkv_sb[:, 0:DIM],
                                                ALU.add)
                        nc.vector.tensor_tensor(V_td[:, t], psV[:], bkv_sb[:, DIM:],
                                                ALU.add)
                    # local memory sums; each accumulation group must be
                    # contiguous matmul instructions
                    for pg, t0 in ((pgA, 0), (pgB, 4)):
                        for h in range(H):
                            hc = slice(h * D, (h + 1) * D)
                            for tt in range(4):
                                nc.tensor.matmul(pg[:, hc], K_td[:, t0 + tt, hc],
                                                 V_td[:, t0 + tt, hc],
                                                 start=(tt == 0), stop=(tt == 3))
                    nc.vector.tensor_copy(GcA_sb[:], pgA[:])
                    nc.vector.tensor_copy(GcB_sb[:], pgB[:])
                    nc.vector.tensor_tensor(Gc_bf[:], GcA_sb[:], GcB_sb[:], ALU.add)
                    nc.sync.dma_start(ag_in[:], Gc_bf[:])

                nc.gpsimd.collective_compute(
                    "AllGather", ALU.bypass,
                    replica_groups=[list(range(N_CORES))],
                    ins=[ag_in.opt()], outs=[ag_out.opt()],
                )

                # ---- stage 2b: Q/K head-block projections ----
                with tc.tile_pool(name="qk", bufs=1) as qkp:
                    QT = qkp.tile([D, H, TSH], BF16)
                    KT = qkp.tile([D, H, TSH], BF16)
                    with tc.tile_pool(name="s2bp", bufs=2, space="PSUM") as s2bp:
                        for j in range(H):
                            for nh in range(2):
                                ncol = slice(nh * 512, (nh + 1) * 512)
                                pqk = s2bp.tile([128, 512], FP32)
                                for a in range(4):
                                    nc.tensor.matmul(pqk[:], qkw_sb[:, a, j],
                                                     hT[:, a, ncol],
                                                     start=(a == 0), stop=(a == 3))
                                nc.scalar.activation(QT[:, j, ncol], pqk[0:D, :],
                                                     ACTF.Identity,
                                                     bias=qkb_sb[0:D, j:j + 1])
                                nc.scalar.activation(KT[:, j, ncol], pqk[D:128, :],
                                                     ACTF.Identity,
                                                     bias=qkb_sb[D:128, j:j + 1])

                    # ---- stage 3: prefix fold + chunked scan ----
                    with tc.tile_pool(name="gt", bufs=1) as gtp, \
                         tc.tile_pool(name="tm", bufs=8) as tmp_pool, \
                         tc.tile_pool(name="pt3", bufs=3, space="PSUM") as pt3, \
                         tc.tile_pool(name="py3", bufs=2, space="PSUM") as py3, \
                         tc.tile_pool(name="pgp", bufs=1, space="PSUM") as pgpp:
                        agg_sb = gtp.tile([128, 4, DIM], BF16)
                        nc.sync.dma_start(
                            agg_sb[:], ag_out[:].rearrange("(a p) m -> p a m", p=128))
                        pgp = pgpp.tile([D, DIM], FP32)
                        for a in range(4):
                            nc.tensor.matmul(pgp[:], sel_sb[:, a], agg_sb[:, a],
                                             start=(a == 0), stop=(a == 3))
                        G0_bf = gtp.tile([D, DIM], BF16)
                        G1_bf = gtp.tile([D, DIM], BF16)
                        nc.vector.tensor_copy(G0_bf[:], pgp[:])
                        nc.vector.tensor_tensor(G1_bf[:], pgp[:], GcA_sb[:], ALU.add)

                        for sc in range(NCH):
                            qcol = slice(sc * 512, (sc + 1) * 512)
                            Gsc = G0_bf if sc == 0 else G1_bf
                            for h in range(H):
                                hc = slice(h * D, (h + 1) * D)
                                Tm = []
                                for bp in range(B):
                                    kcol = slice((sc * 4 + bp) * 128,
                                                 (sc * 4 + bp) * 128 + 128)
                                    pt = pt3.tile([128, 512], FP32)
                                    nc.tensor.matmul(pt[:], KT[:, h, kcol],
                                                     QT[:, h, qcol])
                                    tm = tmp_pool.tile([128, 512], BF16)
                                    nc.vector.tensor_tensor(tm[:], pt[:], mask[:],
                                                            ALU.mult)
                                    Tm.append(tm)
                                py = py3.tile([D, 512], FP32)
                                nc.tensor.matmul(py[:], Gsc[:, hc], QT[:, h, qcol],
                                                 start=True, stop=False)
                                for bp in range(B):
                                    nc.tensor.matmul(py[:], V_td[:, sc * 4 + bp, hc],
                                                     Tm[bp][:],
                                                     start=False, stop=(bp == B - 1))
                                nc.scalar.activation(
                                    yT[(h % 2) * D:(h % 2) * D + D, h // 2, qcol],
                                    py[:], ACTF.Copy)

            # ---- stage 4: Wo + residual + ln2 + transpose ----
            h2nT_p = tc.tile_pool(name="h2nT", bufs=1)
            h2_p = tc.tile_pool(name="h2", bufs=1)
            h2nT = h2nT_p.__enter__().tile([128, 4, TSH], BF16)
            h2_sb = h2_p.__enter__().tile([128, NT, DIM], FP32)
            with tc.tile_pool(name="s4w", bufs=4) as s4w, \
                 tc.tile_pool(name="s4p", bufs=2, space="PSUM") as s4p, \
                 tc.tile_pool(name="s4pt", bufs=2, space="PSUM") as s4pt:
                for t in range(NT):
                    tcol = slice(t * 128, (t + 1) * 128)
                    po = s4p.tile([128, DIM], FP32)
                    for a in range(4):
                        nc.tensor.matmul(po[:], yT[:, a, tcol], woT_sb[:, a],
                                         start=(a == 0), stop=(a == 3))
                    nc.vector.tensor_tensor(h2_sb[:, t], po[:], x_sb[:, t], ALU.add)
                    hn = s4w.tile([128, DIM], BF16, tag="hn2")
                    sq = s4w.tile([128, DIM], BF16, tag="sq2")
                    _ln_normalize(nc, lnp, h2_sb[:, t], hn, sq, eps_sb)
                    for a in range(4):
                        ps = s4pt.tile([128, 128], BF16)
                        nc.tensor.transpose(ps[:], hn[:, a * 128:(a + 1) * 128], identity[:])
                        nc.vector.tensor_copy(h2nT[:, a, t * 128:(t + 1) * 128], ps[:])

            # ---- stage 5: CMS chain ----
            with tc.tile_pool(name="g", bufs=1) as gp, \
                 tc.tile_pool(name="bts", bufs=2) as bts, \
                 tc.tile_pool(name="s5o", bufs=3) as s5o, \
                 tc.tile_pool(name="s5p", bufs=4, space="PSUM") as s5p:
                g_sb = gp.tile([128, 16, TSH], BF16)
                cur = h2nT
                for lvl in range(NLVL):
                    if lvl == 0:
                        w1_sb = w1_sb0
                    else:
                        w1_sb = wts.tile([128, 4, 16, 128], BF16, tag="w1")
                        nc.sync.dma_start(
                            w1_sb[:],
                            w1[lvl].rearrange("p (a h q) -> p a h q", a=4, h=16))
                    b1_sb = bts.tile([128, HID // 128], FP32, tag="b1")
                    nc.sync.dma_start(b1_sb[:], b1[lvl])
                    for ht in range(16):
                        for nh in range(2):
                            colw = slice(nh * 512, nh * 512 + 512)
                            ps = s5p.tile([128, 512], FP32)
                            for a in range(4):
                                nc.tensor.matmul(ps[:], w1_sb[:, a, ht],
                                                 cur[:, a, colw],
                                                 start=(a == 0), stop=(a == 3))
                            nc.scalar.activation(
                                g_sb[:, ht, colw], ps[:], ACTF.Gelu_apprx_tanh,
                                bias=b1_sb[:, ht:ht + 1])
                    if lvl == 0:
                        w2_sb = w2_sb0
                    else:
                        w2_sb = wts.tile([128, 16, 4, 128], BF16, tag="w2")
                        nc.sync.dma_start(
                            w2_sb[:],
                            w2[lvl].rearrange("p (h a q) -> p h a q", h=16, a=4))
                    if lvl < 2:
                        b2_sb = bts.tile([128, 4], FP32, tag="b2")
                        nc.sync.dma_start(b2_sb[:], b2a[lvl])
                        nxt = s5o.tile([128, 4, TSH], BF16, tag="nxt")
                        for a in range(4):
                            for nh in range(2):
                                colw = slice(nh * 512, nh * 512 + 512)
                                ps = s5p.tile([128, 512], FP32)
                                for ht in range(16):
                                    nc.tensor.matmul(ps[:], w2_sb[:, ht, a],
                                                     g_sb[:, ht, colw],
                                                     start=(ht == 0), stop=(ht == 15))
                                nc.scalar.activation(
                                    nxt[:, a, colw], ps[:], ACTF.Identity,
                                    bias=b2_sb[:, a:a + 1])
                        cur = nxt
                    else:
                        # last level emits [tok, dim]; add b2 + residual, write out
                        w2r = w2_sb[:].rearrange("p h a q -> p h (a q)")
                        for t in range(NT):
                            ps = s5p.tile([128, 512], FP32)
                            for ht in range(16):
                                nc.tensor.matmul(
                                    ps[:], g_sb[:, ht, t * 128:(t + 1) * 128],
                                    w2r[:, ht],
                                    start=(ht == 0), stop=(ht == 15))
                            tmp = s5o.tile([128, DIM], FP32, tag="fin")
                            nc.vector.tensor_tensor(tmp[:], ps[:], b2l_sb[:], ALU.add)
                            nc.vector.tensor_tensor(tmp[:], tmp[:], h2_sb[:, t], ALU.add)
                            nc.sync.dma_start(out_t[:, t], tmp[:])
            h2_p.__exit__(None, None, None)
            h2nT_p.__exit__(None, None, None)
            yT_p.__exit__(None, None, None)

    nc.finalize()
    return nc


_NC_CACHE = {}


def _get_nc():
    if "nc" not in _NC_CACHE:
        _NC_CACHE["nc"] = build_kernel()
    return _NC_CACHE["nc"]


def kernel(x, Wq, Wk, Wv, Wo, ln1_w, ln1_b, ln2_w, ln2_b,
           cms_W1, cms_b1, cms_W2, cms_b2):
    bf = ml_dtypes.bfloat16
    f32 = np.float32
    x = np.asarray(x, f32)
    ln1_w = np.asarray(ln1_w, f32); ln1_b = np.asarray(ln1_b, f32)
    ln2_w = np.asarray(ln2_w, f32); ln2_b = np.asarray(ln2_b, f32)

    Wq = np.asarray(Wq, f32); Wk = np.asarray(Wk, f32); Wv = np.asarray(Wv, f32)
    Wo = np.asarray(Wo, f32)
    Wqs = Wq * ln1_w[None, :]; Wks = Wk * ln1_w[None, :]
    Wvs = (Wv * ln1_w[None, :]) / B
    bq = Wq @ ln1_b; bk = Wk @ ln1_b; bv = (Wv @ ln1_b) / B

    W1 = np.asarray(cms_W1, f32).copy(); b1v = np.asarray(cms_b1, f32).copy()
    W2 = np.asarray(cms_W2, f32); b2v = np.asarray(cms_b2, f32)
    b1v[0] = b1v[0] + ln2_b @ W1[0]
    W1[0] = W1[0] * ln2_w[:, None]

    # [128, 4a, 8j, 128m]: column block j = head j's (q|k) output dims
    QK = np.concatenate([Wqs.reshape(H, D, DIM), Wks.reshape(H, D, DIM)], axis=1)
    qkw_arr = QK.transpose(2, 0, 1).reshape(4, 128, H, 128).transpose(1, 0, 2, 3)
    qkb_arr = np.concatenate([bq.reshape(H, D), bk.reshape(H, D)], axis=1).T
    wkT_arr = Wks.T.reshape(4, 128, DIM).transpose(1, 0, 2)
    wvT_arr = Wvs.T.reshape(4, 128, DIM).transpose(1, 0, 2)
    bkv_arr = np.broadcast_to(np.concatenate([bk, bv]), (128, 2 * DIM))
    woT_arr = Wo.T.reshape(4, 128, DIM).transpose(1, 0, 2)

    w1_arr = W1.reshape(NLVL, 4, 128, 16, 128).transpose(0, 2, 1, 3, 4).reshape(
        NLVL, 128, 4 * 16 * 128)
    w2_arr = W2.reshape(NLVL, 16, 128, 4, 128).transpose(0, 2, 1, 3, 4).reshape(
        NLVL, 128, 16 * 4 * 128)
    b1r = np.ascontiguousarray(b1v.reshape(NLVL, HID // 128, 128).transpose(0, 2, 1))
    b2ar = np.ascontiguousarray(b2v[:2].reshape(2, DIM // 128, 128).transpose(0, 2, 1))
    b2lr = np.broadcast_to(b2v[2], (128, DIM)).copy()

    rows = np.arange(512)
    in_maps = []
    for c in range(N_CORES):
        xs = x[:, c * SSH:(c + 1) * SSH, :]            # [4, 256, 512]
        x_tiled = xs.reshape(B, NCH, 128, DIM).transpose(2, 1, 0, 3).reshape(
            128, NT, DIM)
        sel_arr = ((rows // D < c)[:, None] &
                   ((rows % D)[:, None] == np.arange(D)[None, :])).astype(f32)
        sel_t = sel_arr.reshape(4, 128, D).transpose(1, 0, 2)
        in_maps.append({
            "x_t": np.ascontiguousarray(x_tiled),
            "qkw": qkw_arr.astype(bf),
            "qk_b": qkb_arr.astype(f32),
            "wkT": wkT_arr.astype(bf),
            "wvT": wvT_arr.astype(bf),
            "bkv": bkv_arr.astype(f32),
            "woT": woT_arr.astype(bf),
            "sel": sel_t.astype(bf),
            "w1": w1_arr.astype(bf),
            "w2": w2_arr.astype(bf),
            "b1": b1r.astype(f32),
            "b2a": b2ar.astype(f32),
            "b2last": b2lr.astype(f32),
        })

    nc = _get_nc()
    res = run_bass_kernel_spmd(nc, in_maps, core_ids=list(range(N_CORES)))
    _NC_CACHE["last_result"] = res
    out = np.empty((B, S, DIM), dtype=f32)
    for c in range(N_CORES):
        r = res.results[c]["out_t"]                    # [128, 8, 512]
        out[:, c * SSH:(c + 1) * SSH, :] = r.reshape(
            128, NCH, B, DIM).transpose(2, 1, 0, 3).reshape(B, SSH, DIM)
    return out


# revision 15
# speedup vs baseline: 1.5470x; 1.0456x over previous
"""HOPE block kernel for 8 Trainium2 NeuronCores.

Sequence-parallel sharding: core c owns timesteps [256c, 256(c+1)) of all 4
batches (1024 tokens) and computes ALL 8 heads locally.  The linear-attention
memory M = cumsum_t(mean_b v k^T) is split into a local (within-shard) masked
scan plus a cross-core prefix: each core AllGathers its per-shard memory sum
G_c (8 heads x 64x64, 64KB bf16) and folds sum_{c'<c} G_c' in with a
0/1-mask matmul.  Everything else (LN1/QKV/scan/Wo/LN2/CMS) is local.

ln scales/biases and the 1/B factor are folded into the projection weights
host-side; all weights are pre-arranged host-side so every DMA is a plain
[128, contiguous] transfer.
"""

import numpy as np
import ml_dtypes

import concourse.bass as bass
import concourse.bacc as bacc
import concourse.mybir as mybir
import concourse.tile as tile
from concourse.bass_utils import run_bass_kernel_spmd
from concourse.masks import make_identity

N_CORES = 8
B, S, DIM = 4, 2048, 512
H, D = 8, 64
HID = 4 * DIM
NLVL = 3
EPS = 1e-5
SSH = S // N_CORES       # 256 timesteps per core
TSH = B * SSH            # 1024 tokens per core
NT = TSH // 128          # 8 token tiles (tile t = chunk(t//4), batch(t%4))
NCH = SSH // 128         # 2 chunks of 128 steps
FP32 = mybir.dt.float32
BF16 = mybir.dt.bfloat16
AX = mybir.AxisListType.X
ALU = mybir.AluOpType
ACTF = mybir.ActivationFunctionType


def _ln_normalize(nc, pool, xt, out_bf, sq_scratch, eps_tile):
    """out_bf = (xt - mean(xt)) * rsqrt(var(xt)+EPS), per 128-token tile."""
    ssum = pool.tile([128, 1], FP32, tag="ln_s")
    sumsq = pool.tile([128, 1], FP32, tag="ln_q")
    nc.vector.tensor_reduce(ssum[:], xt[:], AX, ALU.add)
    nc.scalar.activation(sq_scratch[:], xt[:], ACTF.Square, accum_out=sumsq[:])
    negmu = pool.tile([128, 1], FP32, tag="ln_m")
    nc.vector.tensor_scalar_mul(negmu[:], ssum[:], -1.0 / DIM)
    e2 = pool.tile([128, 1], FP32, tag="ln_e")
    nc.vector.tensor_scalar_mul(e2[:], sumsq[:], 1.0 / DIM)
    mu2 = pool.tile([128, 1], FP32, tag="ln_2")
    nc.vector.tensor_tensor(mu2[:], negmu[:], negmu[:], ALU.mult)
    var = pool.tile([128, 1], FP32, tag="ln_v")
    nc.vector.tensor_tensor(var[:], e2[:], mu2[:], ALU.subtract)
    std = pool.tile([128, 1], FP32, tag="ln_d")
    nc.scalar.activation(std[:], var[:], ACTF.Sqrt, bias=eps_tile[:])
    rs = pool.tile([128, 1], FP32, tag="ln_r")
    nc.vector.reciprocal(rs[:], std[:])
    nc.vector.tensor_scalar(
        out=out_bf[:], in0=xt[:], scalar1=negmu[:], scalar2=rs[:],
        op0=ALU.add, op1=ALU.mult,
    )


def build_kernel():
    nc = bacc.Bacc(num_devices=N_CORES)

    x_t = nc.dram_tensor("x_t", [128, NT, DIM], FP32, kind="ExternalInput")
    qkw = nc.dram_tensor("qkw", [128, 4, H, 128], BF16, kind="ExternalInput")
    qk_b = nc.dram_tensor("qk_b", [128, H], FP32, kind="ExternalInput")
    wkT = nc.dram_tensor("wkT", [128, 4, DIM], BF16, kind="ExternalInput")
    wvT = nc.dram_tensor("wvT", [128, 4, DIM], BF16, kind="ExternalInput")
    bkv = nc.dram_tensor("bkv", [128, 2 * DIM], FP32, kind="ExternalInput")
    woT = nc.dram_tensor("woT", [128, 4, DIM], BF16, kind="ExternalInput")
    sel = nc.dram_tensor("sel", [128, 4, D], BF16, kind="ExternalInput")
    w1 = nc.dram_tensor("w1", [NLVL, 128, 4 * 16 * 128], BF16, kind="ExternalInput")
    w2 = nc.dram_tensor("w2", [NLVL, 128, 16 * 4 * 128], BF16, kind="ExternalInput")
    b1 = nc.dram_tensor("b1", [NLVL, 128, HID // 128], FP32, kind="ExternalInput")
    b2a = nc.dram_tensor("b2a", [2, 128, DIM // 128], FP32, kind="ExternalInput")
    b2last = nc.dram_tensor("b2last", [128, DIM], FP32, kind="ExternalInput")
    out_t = nc.dram_tensor("out_t", [128, NT, DIM], FP32, kind="ExternalOutput")

    with tile.TileContext(nc) as tc:
        with tc.tile_pool(name="dram", bufs=1, space="DRAM") as dram, \
             tc.tile_pool(name="const", bufs=1) as cpool, \
             tc.tile_pool(name="lns", bufs=4) as lnp, \
             tc.tile_pool(name="xp", bufs=1) as xpool, \
             tc.tile_pool(name="cmsw", bufs=2) as wts:

            ag_in = dram.tile([D, DIM], BF16)
            ag_out = dram.tile([N_CORES * D, DIM], BF16, addr_space="Shared")

            # x first: everything downstream waits on it
            x_sb = xpool.tile([128, NT, DIM], FP32)
            nc.sync.dma_start(x_sb[:], x_t[:])

            identity = cpool.tile([128, 128], BF16)
            make_identity(nc, identity[:])
            # keep-mask tiled over 4 batch column blocks: mask[t, s%128]=1 if t<=s
            mask = cpool.tile([128, 512], FP32)
            nc.gpsimd.memset(mask[:], 1.0)
            for bb in range(4):
                nc.gpsimd.affine_select(
                    out=mask[:, bb * 128:(bb + 1) * 128],
                    in_=mask[:, bb * 128:(bb + 1) * 128],
                    compare_op=ALU.is_ge, fill=0.0,
                    base=0, pattern=[[1, 128]], channel_multiplier=-1,
                )

            qkw_sb = cpool.tile([128, 4, H, 128], BF16)
            nc.sync.dma_start(qkw_sb[:], qkw[:])
            qkb_sb = cpool.tile([128, H], FP32)
            nc.sync.dma_start(qkb_sb[:], qk_b[:])
            wkT_sb = cpool.tile([128, 4, DIM], BF16)
            nc.sync.dma_start(wkT_sb[:], wkT[:])
            wvT_sb = cpool.tile([128, 4, DIM], BF16)
            nc.sync.dma_start(wvT_sb[:], wvT[:])
            bkv_sb = cpool.tile([128, 2 * DIM], FP32)
            nc.sync.dma_start(bkv_sb[:], bkv[:])
            woT_sb = cpool.tile([128, 4, DIM], BF16)
            nc.sync.dma_start(woT_sb[:], woT[:])
            sel_sb = cpool.tile([128, 4, D], BF16)
            nc.sync.dma_start(sel_sb[:], sel[:])
            b2l_sb = cpool.tile([128, DIM], FP32)
            nc.sync.dma_start(b2l_sb[:], b2last[:])
            eps_sb = cpool.tile([128, 1], FP32)
            nc.vector.memset(eps_sb[:], EPS)

            # prefetch CMS level-0 weights (needed only after attention)
            w1_sb0 = wts.tile([128, 4, 16, 128], BF16, tag="w1")
            nc.sync.dma_start(w1_sb0[:], w1[0].rearrange("p (a h q) -> p a h q", a=4, h=16))
            w2_sb0 = wts.tile([128, 16, 4, 128], BF16, tag="w2")
            nc.sync.dma_start(w2_sb0[:], w2[0].rearrange("p (h a q) -> p h a q", h=16, a=4))

            # long-lived attention tiles
            yT_p = tc.tile_pool(name="yTp", bufs=1)
            yT = yT_p.__enter__().tile([128, 4, TSH], BF16)

            with tc.tile_pool(name="hT", bufs=1) as hTp, \
                 tc.tile_pool(name="kv", bufs=1) as kvp, \
                 tc.tile_pool(name="s1w", bufs=3) as s1w:
                hT = hTp.tile([128, 4, TSH], BF16)
                K_td = kvp.tile([128, NT, DIM], BF16)
                V_td = kvp.tile([128, NT, DIM], BF16)
                GcA_sb = kvp.tile([D, DIM], FP32)
                GcB_sb = kvp.tile([D, DIM], FP32)
                Gc_bf = kvp.tile([D, DIM], BF16)

                # ---- stage 1: ln1 per token tile + transpose to [dim, tok] ----
                with tc.tile_pool(name="s1p", bufs=2, space="PSUM") as s1p:
                    for t in range(NT):
                        hn = s1w.tile([128, DIM], BF16, tag="hn")
                        sq = s1w.tile([128, DIM], BF16, tag="sq")
                        _ln_normalize(nc, lnp, x_sb[:, t], hn, sq, eps_sb)
                        for a in range(4):
                            ps = s1p.tile([128, 128], BF16)
                            nc.tensor.transpose(ps[:], hn[:, a * 128:(a + 1) * 128],
                                                identity[:])
                            nc.vector.tensor_copy(hT[:, a, t * 128:(t + 1) * 128], ps[:])

                # ---- stage 2a: token-side K/V projections + local memory sums ----
                with tc.tile_pool(name="s2aw", bufs=2, space="PSUM") as s2ap, \
                     tc.tile_pool(name="s2g", bufs=1, space="PSUM") as s2gp:
                    pgA = s2gp.tile([D, DIM], FP32)
                    pgB = s2gp.tile([D, DIM], FP32)
                    for t in range(NT):
                        tcol = slice(t * 128, (t + 1) * 128)
                        psK = s2ap.tile([128, DIM], FP32, tag="psK")
                        psV = s2ap.tile([128, DIM], FP32, tag="psV")
                        for a in range(4):
                            nc.tensor.matmul(psK[:], hT[:, a, tcol], wkT_sb[:, a],
                                             start=(a == 0), stop=(a == 3))
                        for a in range(4):
                            nc.tensor.matmul(psV[:], hT[:, a, tcol], wvT_sb[:, a],
                                             start=(a == 0), stop=(a == 3))
                        nc.vector.tensor_tensor(K_td[:, t], psK[:], bkv_sb[:, 0:DIM],
                                                ALU.add)
                        nc.vector.tensor_tensor(V_td[:, t], psV[:], bkv_sb[:, DIM:],
                                                ALU.add)
                    # local memory sums; each accumulation group must be
                    # contiguous matmul instructions
                    for pg, t0 in ((pgA, 0), (pgB, 4)):
                        for h in range(H):
                            hc = slice(h * D, (h + 1) * D)
                            for tt in range(4):
                                nc.tensor.matmul(pg[:, hc], K_td[:, t0 + tt, hc],
                                                 V_td[:, t0 + tt, hc],
                                                 start=(tt == 0), stop=(tt == 3))
                    nc.vector.tensor_copy(GcA_sb[:], pgA[:])
                    nc.vector.tensor_copy(GcB_sb[:], pgB[:])
                    nc.vector.tensor_tensor(Gc_bf[:], GcA_sb[:], GcB_sb[:], ALU.add)
                    nc.sync.dma_start(ag_in[:], Gc_bf[:])

                nc.gpsimd.collective_compute(
                    "AllGather", ALU.bypass,
                    replica_groups=[list(range(N_CORES))],
                    ins=[ag_in.opt()], outs=[ag_out.opt()],
                )

                # ---- stage 2b: Q/K head-block projections ----
                with tc.tile_pool(name="qk", bufs=1) as qkp:
                    QT = qkp.tile([D, H, TSH], BF16)
                    KT = qkp.tile([D, H, TSH], BF16)
                    with tc.tile_pool(name="s2bp", bufs=3, space="PSUM") as s2bp:
                        for j in range(H):
                            for nh in range(2):
                                ncol = slice(nh * 512, (nh + 1) * 512)
                                pqk = s2bp.tile([128, 512], FP32)
                                for a in range(4):
                                    nc.tensor.matmul(pqk[:], qkw_sb[:, a, j],
                                                     hT[:, a, ncol],
                                                     start=(a == 0), stop=(a == 3))
                                nc.scalar.activation(QT[:, j, ncol], pqk[0:D, :],
                                                     ACTF.Identity,
                                                     bias=qkb_sb[0:D, j:j + 1])
                                nc.scalar.activation(KT[:, j, ncol], pqk[D:128, :],
                                                     ACTF.Identity,
                                                     bias=qkb_sb[D:128, j:j + 1])

                    # ---- stage 3: prefix fold + chunked scan ----
                    with tc.tile_pool(name="gt", bufs=1) as gtp, \
                         tc.tile_pool(name="tm", bufs=8) as tmp_pool, \
                         tc.tile_pool(name="pt3", bufs=3, space="PSUM") as pt3, \
                         tc.tile_pool(name="py3", bufs=2, space="PSUM") as py3, \
                         tc.tile_pool(name="pgp", bufs=1, space="PSUM") as pgpp:
                        agg_sb = gtp.tile([128, 4, DIM], BF16)
                        nc.sync.dma_start(
                            agg_sb[:], ag_out[:].rearrange("(a p) m -> p a m", p=128))
                        pgp = pgpp.tile([D, DIM], FP32)
                        for a in range(4):
                            nc.tensor.matmul(pgp[:], sel_sb[:, a], agg_sb[:, a],
                                             start=(a == 0), stop=(a == 3))
                        G0_bf = gtp.tile([D, DIM], BF16)
                        G1_bf = gtp.tile([D, DIM], BF16)
                        nc.vector.tensor_copy(G0_bf[:], pgp[:])
                        nc.vector.tensor_tensor(G1_bf[:], pgp[:], GcA_sb[:], ALU.add)

                        # software-pipelined: T/Tm for step n+1 are emitted before
                        # the py group of step n so the PE never waits on the
                        # vector mask-multiply round-trip
                        steps = [(sc, h) for sc in range(NCH) for h in range(H)]

                        def emit_T(sc, h):
                            qcol = slice(sc * 512, (sc + 1) * 512)
                            tms = []
                            for bp in range(B):
                                kcol = slice((sc * 4 + bp) * 128,
                                             (sc * 4 + bp) * 128 + 128)
                                pt = pt3.tile([128, 512], FP32)
                                nc.tensor.matmul(pt[:], KT[:, h, kcol], QT[:, h, qcol])
                                tm = tmp_pool.tile([128, 512], BF16)
                                nc.vector.tensor_tensor(tm[:], pt[:], mask[:], ALU.mult)
                                tms.append(tm)
                            return tms

                        def emit_y(sc, h, tms):
                            qcol = slice(sc * 512, (sc + 1) * 512)
                            hc = slice(h * D, (h + 1) * D)
                            Gsc = G0_bf if sc == 0 else G1_bf
                            py = py3.tile([D, 512], FP32)
                            nc.tensor.matmul(py[:], Gsc[:, hc], QT[:, h, qcol],
                                             start=True, stop=False)
                            for bp in range(B):
                                nc.tensor.matmul(py[:], V_td[:, sc * 4 + bp, hc],
                                                 tms[bp][:],
                                                 start=False, stop=(bp == B - 1))
                            nc.scalar.activation(
                                yT[(h % 2) * D:(h % 2) * D + D, h // 2, qcol],
                                py[:], ACTF.Copy)

                        pending = None
                        for sc, h in steps:
                            tms = emit_T(sc, h)
                            if pending is not None:
                                emit_y(*pending)
                            pending = (sc, h, tms)
                        emit_y(*pending)

            # ---- stage 4: Wo + residual + ln2 + transpose ----
            h2nT_p = tc.tile_pool(name="h2nT", bufs=1)
            h2_p = tc.tile_pool(name="h2", bufs=1)
            h2nT = h2nT_p.__enter__().tile([128, 4, TSH], BF16)
            h2_sb = h2_p.__enter__().tile([128, NT, DIM], FP32)
            with tc.tile_pool(name="s4w", bufs=4) as s4w, \
                 tc.tile_pool(name="s4p", bufs=2, space="PSUM") as s4p, \
                 tc.tile_pool(name="s4pt", bufs=2, space="PSUM") as s4pt:
                for t in range(NT):
                    tcol = slice(t * 128, (t + 1) * 128)
                    po = s4p.tile([128, DIM], FP32)
                    for a in range(4):
                        nc.tensor.matmul(po[:], yT[:, a, tcol], woT_sb[:, a],
                                         start=(a == 0), stop=(a == 3))
                    nc.vector.tensor_tensor(h2_sb[:, t], po[:], x_sb[:, t], ALU.add)
                    hn = s4w.tile([128, DIM], BF16, tag="hn2")
                    sq = s4w.tile([128, DIM], BF16, tag="sq2")
                    _ln_normalize(nc, lnp, h2_sb[:, t], hn, sq, eps_sb)
                    for a in range(4):
                        ps = s4pt.tile([128, 128], BF16)
                        nc.tensor.transpose(ps[:], hn[:, a * 128:(a + 1) * 128], identity[:])
                        nc.vector.tensor_copy(h2nT[:, a, t * 128:(t + 1) * 128], ps[:])

            # ---- stage 5: CMS chain ----
            with tc.tile_pool(name="g", bufs=1) as gp, \
                 tc.tile_pool(name="bts", bufs=2) as bts, \
                 tc.tile_pool(name="s5o", bufs=3) as s5o, \
                 tc.tile_pool(name="s5p", bufs=4, space="PSUM") as s5p:
                g_sb = gp.tile([128, 16, TSH], BF16)
                cur = h2nT
                for lvl in range(NLVL):
                    if lvl == 0:
                        w1_sb = w1_sb0
                    else:
                        w1_sb = wts.tile([128, 4, 16, 128], BF16, tag="w1")
                        nc.sync.dma_start(
                            w1_sb[:],
                            w1[lvl].rearrange("p (a h q) -> p a h q", a=4, h=16))
                    b1_sb = bts.tile([128, HID // 128], FP32, tag="b1")
                    nc.sync.dma_start(b1_sb[:], b1[lvl])
                    for ht in range(16):
                        for nh in range(2):
                            colw = slice(nh * 512, nh * 512 + 512)
                            ps = s5p.tile([128, 512], FP32)
                            for a in range(4):
                                nc.tensor.matmul(ps[:], w1_sb[:, a, ht],
                                                 cur[:, a, colw],
                                                 start=(a == 0), stop=(a == 3))
                            nc.scalar.activation(
                                g_sb[:, ht, colw], ps[:], ACTF.Gelu_apprx_tanh,
                                bias=b1_sb[:, ht:ht + 1])
                    if lvl == 0:
                        w2_sb = w2_sb0
                    else:
                        w2_sb = wts.tile([128, 16, 4, 128], BF16, tag="w2")
                        nc.sync.dma_start(
                            w2_sb[:],
                            w2[lvl].rearrange("p (h a q) -> p h a q", h=16, a=4))
                    if lvl < 2:
                        b2_sb = bts.tile([128, 4], FP32, tag="b2")
                        nc.sync.dma_start(b2_sb[:], b2a[lvl])
                        nxt = s5o.tile([128, 4, TSH], BF16, tag="nxt")
                        for a in range(4):
                            for nh in range(2):
                                colw = slice(nh * 512, nh * 512 + 512)
                                ps = s5p.tile([128, 512], FP32)
                                for ht in range(16):
                                    nc.tensor.matmul(ps[:], w2_sb[:, ht, a],
                                                     g_sb[:, ht, colw],
                                                     start=(ht == 0), stop=(ht == 15))
                                nc.scalar.activation(
                                    nxt[:, a, colw], ps[:], ACTF.Identity,
                                    bias=b2_sb[:, a:a + 1])
                        cur = nxt
                    else:
                        # last level emits [tok, dim]; add b2 + residual, write out
                        w2r = w2_sb[:].rearrange("p h a q -> p h (a q)")
                        for t in range(NT):
                            ps = s5p.tile([128, 512], FP32)
                            for ht in range(16):
                                nc.tensor.matmul(
                                    ps[:], g_sb[:, ht, t * 128:(t + 1) * 128],
                                    w2r[:, ht],
                                    start=(ht == 0), stop=(ht == 15))
                            tmp = s5o.tile([128, DIM], FP32, tag="fin")
                            nc.vector.tensor_tensor(tmp[:], ps[:], b2l_sb[:], ALU.add)
                            nc.vector.tensor_tensor(tmp[:], tmp[:], h2_sb[:, t], ALU.add)
                            nc.sync.dma_start(out_t[:, t], tmp[:])
            h2_p.__exit__(None, None, None)
            h2nT_p.__exit__(None, None, None)
            yT_p.__exit__(None, None, None)

    nc.finalize()
    return nc


_NC_CACHE = {}


def _get_nc():
    if "nc" not in _NC_CACHE:
        _NC_CACHE["nc"] = build_kernel()
    return _NC_CACHE["nc"]


def kernel(x, Wq, Wk, Wv, Wo, ln1_w, ln1_b, ln2_w, ln2_b,
           cms_W1, cms_b1, cms_W2, cms_b2):
    bf = ml_dtypes.bfloat16
    f32 = np.float32
    x = np.asarray(x, f32)
    ln1_w = np.asarray(ln1_w, f32); ln1_b = np.asarray(ln1_b, f32)
    ln2_w = np.asarray(ln2_w, f32); ln2_b = np.asarray(ln2_b, f32)

    Wq = np.asarray(Wq, f32); Wk = np.asarray(Wk, f32); Wv = np.asarray(Wv, f32)
    Wo = np.asarray(Wo, f32)
    Wqs = Wq * ln1_w[None, :]; Wks = Wk * ln1_w[None, :]
    Wvs = (Wv * ln1_w[None, :]) / B
    bq = Wq @ ln1_b; bk = Wk @ ln1_b; bv = (Wv @ ln1_b) / B

    W1 = np.asarray(cms_W1, f32).copy(); b1v = np.asarray(cms_b1, f32).copy()
    W2 = np.asarray(cms_W2, f32); b2v = np.asarray(cms_b2, f32)
    b1v[0] = b1v[0] + ln2_b @ W1[0]
    W1[0] = W1[0] * ln2_w[:, None]

    # [128, 4a, 8j, 128m]: column block j = head j's (q|k) output dims
    QK = np.concatenate([Wqs.reshape(H, D, DIM), Wks.reshape(H, D, DIM)], axis=1)
    qkw_arr = QK.transpose(2, 0, 1).reshape(4, 128, H, 128).transpose(1, 0, 2, 3)
    qkb_arr = np.concatenate([bq.reshape(H, D), bk.reshape(H, D)], axis=1).T
    wkT_arr = Wks.T.reshape(4, 128, DIM).transpose(1, 0, 2)
    wvT_arr = Wvs.T.reshape(4, 128, DIM).transpose(1, 0, 2)
    bkv_arr = np.broadcast_to(np.concatenate([bk, bv]), (128, 2 * DIM))
    woT_arr = Wo.T.reshape(4, 128, DIM).transpose(1, 0, 2)

    w1_arr = W1.reshape(NLVL, 4, 128, 16, 128).transpose(0, 2, 1, 3, 4).reshape(
        NLVL, 128, 4 * 16 * 128)
    w2_arr = W2.reshape(NLVL, 16, 128, 4, 128).transpose(0, 2, 1, 3, 4).reshape(
        NLVL, 128, 16 * 4 * 128)
    b1r = np.ascontiguousarray(b1v.reshape(NLVL, HID // 128, 128).transpose(0, 2, 1))
    b2ar = np.ascontiguousarray(b2v[:2].reshape(2, DIM // 128, 128).transpose(0, 2, 1))
    b2lr = np.broadcast_to(b2v[2], (128, DIM)).copy()

    rows = np.arange(512)
    in_maps = []
    for c in range(N_CORES):
        xs = x[:, c * SSH:(c + 1) * SSH, :]            # [4, 256, 512]
        x_tiled = xs.reshape(B, NCH, 128, DIM).transpose(2, 1, 0, 3).reshape(
            128, NT, DIM)
        sel_arr = ((rows // D < c)[:, None] &
                   ((rows % D)[:, None] == np.arange(D)[None, :])).astype(f32)
        sel_t = sel_arr.reshape(4, 128, D).transpose(1, 0, 2)
        in_maps.append({
            "x_t": np.ascontiguousarray(x_tiled),
            "qkw": qkw_arr.astype(bf),
            "qk_b": qkb_arr.astype(f32),
            "wkT": wkT_arr.astype(bf),
            "wvT": wvT_arr.astype(bf),
            "bkv": bkv_arr.astype(f32),
            "woT": woT_arr.astype(bf),
            "sel": sel_t.astype(bf),
            "w1": w1_arr.astype(bf),
            "w2": w2_arr.astype(bf),
            "b1": b1r.astype(f32),
            "b2a": b2ar.astype(f32),
            "b2last": b2lr.astype(f32),
        })

    nc = _get_nc()
    res = run_bass_kernel_spmd(nc, in_maps, core_ids=list(range(N_CORES)))
    _NC_CACHE["last_result"] = res
    out = np.empty((B, S, DIM), dtype=f32)
    for c in range(N_CORES):
        r = res.results[c]["out_t"]                    # [128, 8, 512]
        out[:, c * SSH:(c + 1) * SSH, :] = r.reshape(
            128, NCH, B, DIM).transpose(2, 1, 0, 3).reshape(B, SSH, DIM)
    return out


# revision 20
# speedup vs baseline: 1.6679x; 1.0781x over previous
"""HOPE block kernel for 8 Trainium2 NeuronCores.

Sequence-parallel sharding: core c owns timesteps [256c, 256(c+1)) of all 4
batches (1024 tokens) and computes ALL 8 heads locally.  The linear-attention
memory M = cumsum_t(mean_b v k^T) is split into a local (within-shard) masked
scan plus a cross-core prefix: each core AllGathers its per-shard memory sum
G_c (8 heads x 64x64, 64KB bf16) and folds sum_{c'<c} G_c' in with a
0/1-mask matmul.  Everything else (LN1/QKV/scan/Wo/LN2/CMS) is local.

ln scales/biases and the 1/B factor are folded into the projection weights
host-side; all weights are pre-arranged host-side so every DMA is a plain
[128, contiguous] transfer.
"""

import numpy as np
import ml_dtypes

import concourse.bass as bass
import concourse.bacc as bacc
import concourse.mybir as mybir
import concourse.tile as tile
from concourse.bass_utils import run_bass_kernel_spmd
from concourse.masks import make_identity

N_CORES = 8
B, S, DIM = 4, 2048, 512
H, D = 8, 64
HID = 4 * DIM
NLVL = 3
EPS = 1e-5
SSH = S // N_CORES       # 256 timesteps per core
TSH = B * SSH            # 1024 tokens per core
NT = TSH // 128          # 8 token tiles (tile t = chunk(t//4), batch(t%4))
NCH = SSH // 128         # 2 chunks of 128 steps
FP32 = mybir.dt.float32
BF16 = mybir.dt.bfloat16
AX = mybir.AxisListType.X
ALU = mybir.AluOpType
ACTF = mybir.ActivationFunctionType


def _ln_normalize(nc, pool, xt, out_bf, sq_scratch, eps_tile):
    """out_bf = (xt - mean(xt)) * rsqrt(var(xt)+EPS), per 128-token tile."""
    ssum = pool.tile([128, 1], FP32, tag="ln_s")
    sumsq = pool.tile([128, 1], FP32, tag="ln_q")
    nc.vector.tensor_reduce(ssum[:], xt[:], AX, ALU.add)
    nc.scalar.activation(sq_scratch[:], xt[:], ACTF.Square, accum_out=sumsq[:])
    negmu = pool.tile([128, 1], FP32, tag="ln_m")
    nc.vector.tensor_scalar_mul(negmu[:], ssum[:], -1.0 / DIM)
    e2 = pool.tile([128, 1], FP32, tag="ln_e")
    nc.vector.tensor_scalar_mul(e2[:], sumsq[:], 1.0 / DIM)
    mu2 = pool.tile([128, 1], FP32, tag="ln_2")
    nc.vector.tensor_tensor(mu2[:], negmu[:], negmu[:], ALU.mult)
    var = pool.tile([128, 1], FP32, tag="ln_v")
    nc.vector.tensor_tensor(var[:], e2[:], mu2[:], ALU.subtract)
    std = pool.tile([128, 1], FP32, tag="ln_d")
    nc.scalar.activation(std[:], var[:], ACTF.Sqrt, bias=eps_tile[:])
    rs = pool.tile([128, 1], FP32, tag="ln_r")
    nc.vector.reciprocal(rs[:], std[:])
    nc.vector.tensor_scalar(
        out=out_bf[:], in0=xt[:], scalar1=negmu[:], scalar2=rs[:],
        op0=ALU.add, op1=ALU.mult,
    )


def build_kernel():
    nc = bacc.Bacc(num_devices=N_CORES)

    x_t = nc.dram_tensor("x_t", [128, NT, DIM], FP32, kind="ExternalInput")
    qkw = nc.dram_tensor("qkw", [128, 4, H, 128], BF16, kind="ExternalInput")
    qk_b = nc.dram_tensor("qk_b", [128, H], FP32, kind="ExternalInput")
    wkT = nc.dram_tensor("wkT", [128, 4, DIM], BF16, kind="ExternalInput")
    wvT = nc.dram_tensor("wvT", [128, 4, DIM], BF16, kind="ExternalInput")
    bkv = nc.dram_tensor("bkv", [128, 2 * DIM], FP32, kind="ExternalInput")
    woT = nc.dram_tensor("woT", [128, 4, DIM], BF16, kind="ExternalInput")
    sel = nc.dram_tensor("sel", [128, 4, D], BF16, kind="ExternalInput")
    w1 = nc.dram_tensor("w1", [NLVL, 128, 4 * 16 * 128], BF16, kind="ExternalInput")
    w2 = nc.dram_tensor("w2", [NLVL, 128, 16 * 4 * 128], BF16, kind="ExternalInput")
    b1 = nc.dram_tensor("b1", [NLVL, 128, HID // 128], FP32, kind="ExternalInput")
    b2a = nc.dram_tensor("b2a", [2, 128, DIM // 128], FP32, kind="ExternalInput")
    b2last = nc.dram_tensor("b2last", [128, DIM], FP32, kind="ExternalInput")
    out_t = nc.dram_tensor("out_t", [128, NT, DIM], FP32, kind="ExternalOutput")

    with tile.TileContext(nc) as tc:
        with tc.tile_pool(name="dram", bufs=1, space="DRAM") as dram, \
             tc.tile_pool(name="const", bufs=1) as cpool, \
             tc.tile_pool(name="lns", bufs=4) as lnp, \
             tc.tile_pool(name="xp", bufs=1) as xpool:

            ag_in = dram.tile([D, DIM], BF16)
            ag_out = dram.tile([N_CORES * D, DIM], BF16, addr_space="Shared")

            # x first: everything downstream waits on it
            x_sb = xpool.tile([128, NT, DIM], FP32)
            nc.sync.dma_start(x_sb[:], x_t[:])

            identity = cpool.tile([128, 128], BF16)
            make_identity(nc, identity[:])
            # keep-mask tiled over 4 batch column blocks: mask[t, s%128]=1 if t<=s
            mask = cpool.tile([128, 512], FP32)
            nc.gpsimd.memset(mask[:], 1.0)
            for bb in range(4):
                nc.gpsimd.affine_select(
                    out=mask[:, bb * 128:(bb + 1) * 128],
                    in_=mask[:, bb * 128:(bb + 1) * 128],
                    compare_op=ALU.is_ge, fill=0.0,
                    base=0, pattern=[[1, 128]], channel_multiplier=-1,
                )

            qkw_sb = cpool.tile([128, 4, H, 128], BF16)
            nc.sync.dma_start(qkw_sb[:], qkw[:])
            qkb_sb = cpool.tile([128, H], FP32)
            nc.sync.dma_start(qkb_sb[:], qk_b[:])
            wkT_sb = cpool.tile([128, 4, DIM], BF16)
            nc.sync.dma_start(wkT_sb[:], wkT[:])
            wvT_sb = cpool.tile([128, 4, DIM], BF16)
            nc.sync.dma_start(wvT_sb[:], wvT[:])
            bkv_sb = cpool.tile([128, 2 * DIM], FP32)
            nc.sync.dma_start(bkv_sb[:], bkv[:])
            woT_sb = cpool.tile([128, 4, DIM], BF16)
            nc.sync.dma_start(woT_sb[:], woT[:])
            sel_sb = cpool.tile([128, 4, D], BF16)
            nc.sync.dma_start(sel_sb[:], sel[:])
            b2l_sb = cpool.tile([128, DIM], FP32)
            nc.sync.dma_start(b2l_sb[:], b2last[:])
            eps_sb = cpool.tile([128, 1], FP32)
            nc.vector.memset(eps_sb[:], EPS)

            # long-lived attention tiles
            yT_p = tc.tile_pool(name="yTp", bufs=1)
            yT = yT_p.__enter__().tile([128, 4, TSH], BF16)

            with tc.tile_pool(name="kv", bufs=1) as kvp, \
                 tc.tile_pool(name="qk", bufs=1) as qkp, \
                 tc.tile_pool(name="s1w", bufs=3) as s1w:
                QT = qkp.tile([D, H, TSH], BF16)
                KT = qkp.tile([D, H, TSH], BF16)
                K_td = kvp.tile([128, NT, DIM], BF16)
                V_td = kvp.tile([128, NT, DIM], BF16)
                GcA_sb = kvp.tile([D, DIM], FP32)
                GcB_sb = kvp.tile([D, DIM], FP32)
                Gc_bf = kvp.tile([D, DIM], BF16)

                hT_ctx = tc.tile_pool(name="hT", bufs=1)
                hTp = hT_ctx.__enter__()
                hT = hTp.tile([128, 4, TSH], BF16)
                # ---- stage 1: ln1 per token tile + transpose to [dim, tok] ----
                with tc.tile_pool(name="s1p", bufs=2, space="PSUM") as s1p:
                    for t in range(NT):
                        hn = s1w.tile([128, DIM], BF16, tag="hn")
                        sq = s1w.tile([128, DIM], BF16, tag="sq")
                        _ln_normalize(nc, lnp, x_sb[:, t], hn, sq, eps_sb)
                        for a in range(4):
                            ps = s1p.tile([128, 128], BF16)
                            nc.tensor.transpose(ps[:], hn[:, a * 128:(a + 1) * 128],
                                                identity[:])
                            nc.vector.tensor_copy(hT[:, a, t * 128:(t + 1) * 128], ps[:])

                # ---- stage 2a: token-side K/V projections + local memory sums ----
                with tc.tile_pool(name="s2aw", bufs=2, space="PSUM") as s2ap, \
                     tc.tile_pool(name="s2g", bufs=1, space="PSUM") as s2gp:
                    pgA = s2gp.tile([D, DIM], FP32)
                    pgB = s2gp.tile([D, DIM], FP32)
                    for t in range(NT):
                        tcol = slice(t * 128, (t + 1) * 128)
                        psK = s2ap.tile([128, DIM], FP32, tag="psK")
                        psV = s2ap.tile([128, DIM], FP32, tag="psV")
                        for a in range(4):
                            nc.tensor.matmul(psK[:], hT[:, a, tcol], wkT_sb[:, a],
                                             start=(a == 0), stop=(a == 3))
                        for a in range(4):
                            nc.tensor.matmul(psV[:], hT[:, a, tcol], wvT_sb[:, a],
                                             start=(a == 0), stop=(a == 3))
                        nc.vector.tensor_tensor(K_td[:, t], psK[:], bkv_sb[:, 0:DIM],
                                                ALU.add)
                        nc.vector.tensor_tensor(V_td[:, t], psV[:], bkv_sb[:, DIM:],
                                                ALU.add)
                    # local memory sums; each accumulation group must be
                    # contiguous matmul instructions
                    for pg, t0 in ((pgA, 0), (pgB, 4)):
                        for h in range(H):
                            hc = slice(h * D, (h + 1) * D)
                            for tt in range(4):
                                nc.tensor.matmul(pg[:, hc], K_td[:, t0 + tt, hc],
                                                 V_td[:, t0 + tt, hc],
                                                 start=(tt == 0), stop=(tt == 3))
                    nc.vector.tensor_copy(GcA_sb[:], pgA[:])
                    nc.vector.tensor_copy(GcB_sb[:], pgB[:])
                    nc.vector.tensor_tensor(Gc_bf[:], GcA_sb[:], GcB_sb[:], ALU.add)
                    nc.sync.dma_start(ag_in[:], Gc_bf[:])

                nc.gpsimd.collective_compute(
                    "AllGather", ALU.bypass,
                    replica_groups=[list(range(N_CORES))],
                    ins=[ag_in.opt()], outs=[ag_out.opt()],
                )

                # ---- stage 2b: Q/K head-block projections, with the
                # causal T = mask o (K^T Q) matmuls software-pipelined in:
                # emit QK(j) then T(j-1), so T never blocks the PE ----
                tm_ctx = tc.tile_pool(name="tm", bufs=64)
                tmp_pool = tm_ctx.__enter__()
                tms = {}
                with tc.tile_pool(name="s2bp", bufs=3, space="PSUM") as s2bp, \
                     tc.tile_pool(name="pt3", bufs=4, space="PSUM") as pt3:
                    def emit_qk(j):
                        for nh in range(2):
                            ncol = slice(nh * 512, (nh + 1) * 512)
                            pqk = s2bp.tile([128, 512], FP32)
                            for a in range(4):
                                nc.tensor.matmul(pqk[:], qkw_sb[:, a, j],
                                                 hT[:, a, ncol],
                                                 start=(a == 0), stop=(a == 3))
                            nc.scalar.activation(QT[:, j, ncol], pqk[0:D, :],
                                                 ACTF.Identity,
                                                 bias=qkb_sb[0:D, j:j + 1])
                            nc.scalar.activation(KT[:, j, ncol], pqk[D:128, :],
                                                 ACTF.Identity,
                                                 bias=qkb_sb[D:128, j:j + 1])

                    def emit_T(j):
                        for sc in range(NCH):
                            qcol = slice(sc * 512, (sc + 1) * 512)
                            for bp in range(B):
                                kcol = slice((sc * 4 + bp) * 128,
                                             (sc * 4 + bp) * 128 + 128)
                                pt = pt3.tile([128, 512], FP32)
                                nc.tensor.matmul(pt[:], KT[:, j, kcol],
                                                 QT[:, j, qcol])
                                tm = tmp_pool.tile([128, 512], BF16)
                                nc.vector.tensor_tensor(tm[:], pt[:], mask[:],
                                                        ALU.mult)
                                tms[(sc, j, bp)] = tm

                    for j in range(H + 1):
                        if j < H:
                            emit_qk(j)
                        if j > 0:
                            emit_T(j - 1)

                if True:
                    # ---- stage 3: prefix fold + readout ----
                    with tc.tile_pool(name="gt", bufs=1) as gtp, \
                         tc.tile_pool(name="py3", bufs=2, space="PSUM") as py3, \
                         tc.tile_pool(name="pgp", bufs=1, space="PSUM") as pgpp:
                        agg_sb = gtp.tile([128, 4, DIM], BF16)
                        nc.sync.dma_start(
                            agg_sb[:], ag_out[:].rearrange("(a p) m -> p a m", p=128))
                        pgp = pgpp.tile([D, DIM], FP32)
                        for a in range(4):
                            nc.tensor.matmul(pgp[:], sel_sb[:, a], agg_sb[:, a],
                                             start=(a == 0), stop=(a == 3))
                        G0_bf = gtp.tile([D, DIM], BF16)
                        G1_bf = gtp.tile([D, DIM], BF16)
                        nc.vector.tensor_copy(G0_bf[:], pgp[:])
                        nc.vector.tensor_tensor(G1_bf[:], pgp[:], GcA_sb[:], ALU.add)

                        for sc in range(NCH):
                            qcol = slice(sc * 512, (sc + 1) * 512)
                            Gsc = G0_bf if sc == 0 else G1_bf
                            for h in range(H):
                                hc = slice(h * D, (h + 1) * D)
                                py = py3.tile([D, 512], FP32)
                                nc.tensor.matmul(py[:], Gsc[:, hc], QT[:, h, qcol],
                                                 start=True, stop=False)
                                for bp in range(B):
                                    nc.tensor.matmul(py[:], V_td[:, sc * 4 + bp, hc],
                                                     tms[(sc, h, bp)][:],
                                                     start=False, stop=(bp == B - 1))
                                nc.scalar.activation(
                                    yT[(h % 2) * D:(h % 2) * D + D, h // 2, qcol],
                                    py[:], ACTF.Copy)
                    tm_ctx.__exit__(None, None, None)
                    hT_ctx.__exit__(None, None, None)

            # CMS weight pool + level-0 prefetch (SBUF freed by attention)
            wts_ctx = tc.tile_pool(name="cmsw", bufs=2)
            wts = wts_ctx.__enter__()
            w1_sb0 = wts.tile([128, 4, 16, 128], BF16, tag="w1")
            nc.sync.dma_start(w1_sb0[:], w1[0].rearrange("p (a h q) -> p a h q", a=4, h=16))
            w2_sb0 = wts.tile([128, 16, 4, 128], BF16, tag="w2")
            nc.sync.dma_start(w2_sb0[:], w2[0].rearrange("p (h a q) -> p h a q", h=16, a=4))

            # ---- stage 4: Wo + residual + ln2 + transpose ----
            h2nT_p = tc.tile_pool(name="h2nT", bufs=1)
            h2_p = tc.tile_pool(name="h2", bufs=1)
            h2nT = h2nT_p.__enter__().tile([128, 4, TSH], BF16)
            h2_sb = h2_p.__enter__().tile([128, NT, DIM], FP32)
            with tc.tile_pool(name="s4w", bufs=4) as s4w, \
                 tc.tile_pool(name="s4p", bufs=2, space="PSUM") as s4p, \
                 tc.tile_pool(name="s4pt", bufs=2, space="PSUM") as s4pt:
                for t in range(NT):
                    tcol = slice(t * 128, (t + 1) * 128)
                    po = s4p.tile([128, DIM], FP32)
                    for a in range(4):
                        nc.tensor.matmul(po[:], yT[:, a, tcol], woT_sb[:, a],
                                         start=(a == 0), stop=(a == 3))
                    nc.vector.tensor_tensor(h2_sb[:, t], po[:], x_sb[:, t], ALU.add)
                    hn = s4w.tile([128, DIM], BF16, tag="hn2")
                    sq = s4w.tile([128, DIM], BF16, tag="sq2")
                    _ln_normalize(nc, lnp, h2_sb[:, t], hn, sq, eps_sb)
                    for a in range(4):
                        ps = s4pt.tile([128, 128], BF16)
                        nc.tensor.transpose(ps[:], hn[:, a * 128:(a + 1) * 128], identity[:])
                        nc.vector.tensor_copy(h2nT[:, a, t * 128:(t + 1) * 128], ps[:])

            # ---- stage 5: CMS chain ----
            with tc.tile_pool(name="g", bufs=1) as gp, \
                 tc.tile_pool(name="bts", bufs=2) as bts, \
                 tc.tile_pool(name="s5o", bufs=3) as s5o, \
                 tc.tile_pool(name="s5p", bufs=4, space="PSUM") as s5p:
                g_sb = gp.tile([128, 16, TSH], BF16)
                cur = h2nT
                for lvl in range(NLVL):
                    if lvl == 0:
                        w1_sb = w1_sb0
                    else:
                        w1_sb = wts.tile([128, 4, 16, 128], BF16, tag="w1")
                        nc.sync.dma_start(
                            w1_sb[:],
                            w1[lvl].rearrange("p (a h q) -> p a h q", a=4, h=16))
                    b1_sb = bts.tile([128, HID // 128], FP32, tag="b1")
                    nc.sync.dma_start(b1_sb[:], b1[lvl])
                    for ht in range(16):
                        for nh in range(2):
                            colw = slice(nh * 512, nh * 512 + 512)
                            ps = s5p.tile([128, 512], FP32)
                            for a in range(4):
                                nc.tensor.matmul(ps[:], w1_sb[:, a, ht],
                                                 cur[:, a, colw],
                                                 start=(a == 0), stop=(a == 3))
                            nc.scalar.activation(
                                g_sb[:, ht, colw], ps[:], ACTF.Gelu_apprx_tanh,
                                bias=b1_sb[:, ht:ht + 1])
                    if lvl == 0:
                        w2_sb = w2_sb0
                    else:
                        w2_sb = wts.tile([128, 16, 4, 128], BF16, tag="w2")
                        nc.sync.dma_start(
                            w2_sb[:],
                            w2[lvl].rearrange("p (h a q) -> p h a q", h=16, a=4))
                    if lvl < 2:
                        b2_sb = bts.tile([128, 4], FP32, tag="b2")
                        nc.sync.dma_start(b2_sb[:], b2a[lvl])
                        nxt = s5o.tile([128, 4, TSH], BF16, tag="nxt")
                        for a in range(4):
                            for nh in range(2):
                                colw = slice(nh * 512, nh * 512 + 512)
                                ps = s5p.tile([128, 512], FP32)
                                for ht in range(16):
                                    nc.tensor.matmul(ps[:], w2_sb[:, ht, a],
                                                     g_sb[:, ht, colw],
                                                     start=(ht == 0), stop=(ht == 15))
                                nc.scalar.activation(
                                    nxt[:, a, colw], ps[:], ACTF.Identity,
                                    bias=b2_sb[:, a:a + 1])
                        cur = nxt
                    else:
                        # last level emits [tok, dim]; add b2 + residual, write out
                        w2r = w2_sb[:].rearrange("p h a q -> p h (a q)")
                        for t in range(NT):
                            ps = s5p.tile([128, 512], FP32)
                            for ht in range(16):
                                nc.tensor.matmul(
                                    ps[:], g_sb[:, ht, t * 128:(t + 1) * 128],
                                    w2r[:, ht],
                                    start=(ht == 0), stop=(ht == 15))
                            tmp = s5o.tile([128, DIM], FP32, tag="fin")
                            nc.vector.tensor_tensor(tmp[:], ps[:], b2l_sb[:], ALU.add)
                            nc.vector.tensor_tensor(tmp[:], tmp[:], h2_sb[:, t], ALU.add)
                            nc.sync.dma_start(out_t[:, t], tmp[:])
            h2_p.__exit__(None, None, None)
            h2nT_p.__exit__(None, None, None)
            wts_ctx.__exit__(None, None, None)
            yT_p.__exit__(None, None, None)

    nc.finalize()
    return nc


_NC_CACHE = {}


def _get_nc():
    if "nc" not in _NC_CACHE:
        _NC_CACHE["nc"] = build_kernel()
    return _NC_CACHE["nc"]


def kernel(x, Wq, Wk, Wv, Wo, ln1_w, ln1_b, ln2_w, ln2_b,
           cms_W1, cms_b1, cms_W2, cms_b2):
    bf = ml_dtypes.bfloat16
    f32 = np.float32
    x = np.asarray(x, f32)
    ln1_w = np.asarray(ln1_w, f32); ln1_b = np.asarray(ln1_b, f32)
    ln2_w = np.asarray(ln2_w, f32); ln2_b = np.asarray(ln2_b, f32)

    Wq = np.asarray(Wq, f32); Wk = np.asarray(Wk, f32); Wv = np.asarray(Wv, f32)
    Wo = np.asarray(Wo, f32)
    Wqs = Wq * ln1_w[None, :]; Wks = Wk * ln1_w[None, :]
    Wvs = (Wv * ln1_w[None, :]) / B
    bq = Wq @ ln1_b; bk = Wk @ ln1_b; bv = (Wv @ ln1_b) / B

    W1 = np.asarray(cms_W1, f32).copy(); b1v = np.asarray(cms_b1, f32).copy()
    W2 = np.asarray(cms_W2, f32); b2v = np.asarray(cms_b2, f32)
    b1v[0] = b1v[0] + ln2_b @ W1[0]
    W1[0] = W1[0] * ln2_w[:, None]

    # [128, 4a, 8j, 128m]: column block j = head j's (q|k) output dims
    QK = np.concatenate([Wqs.reshape(H, D, DIM), Wks.reshape(H, D, DIM)], axis=1)
    qkw_arr = QK.transpose(2, 0, 1).reshape(4, 128, H, 128).transpose(1, 0, 2, 3)
    qkb_arr = np.concatenate([bq.reshape(H, D), bk.reshape(H, D)], axis=1).T
    wkT_arr = Wks.T.reshape(4, 128, DIM).transpose(1, 0, 2)
    wvT_arr = Wvs.T.reshape(4, 128, DIM).transpose(1, 0, 2)
    bkv_arr = np.broadcast_to(np.concatenate([bk, bv]), (128, 2 * DIM))
    woT_arr = Wo.T.reshape(4, 128, DIM).transpose(1, 0, 2)

    w1_arr = W1.reshape(NLVL, 4, 128, 16, 128).transpose(0, 2, 1, 3, 4).reshape(
        NLVL, 128, 4 * 16 * 128)
    w2_arr = W2.reshape(NLVL, 16, 128, 4, 128).transpose(0, 2, 1, 3, 4).reshape(
        NLVL, 128, 16 * 4 * 128)
    b1r = np.ascontiguousarray(b1v.reshape(NLVL, HID // 128, 128).transpose(0, 2, 1))
    b2ar = np.ascontiguousarray(b2v[:2].reshape(2, DIM // 128, 128).transpose(0, 2, 1))
    b2lr = np.broadcast_to(b2v[2], (128, DIM)).copy()

    rows = np.arange(512)
    in_maps = []
    for c in range(N_CORES):
        xs = x[:, c * SSH:(c + 1) * SSH, :]            # [4, 256, 512]
        x_tiled = xs.reshape(B, NCH, 128, DIM).transpose(2, 1, 0, 3).reshape(
            128, NT, DIM)
        sel_arr = ((rows // D < c)[:, None] &
                   ((rows % D)[:, None] == np.arange(D)[None, :])).astype(f32)
        sel_t = sel_arr.reshape(4, 128, D).transpose(1, 0, 2)
        in_maps.append({
            "x_t": np.ascontiguousarray(x_tiled),
            "qkw": qkw_arr.astype(bf),
            "qk_b": qkb_arr.astype(f32),
            "wkT": wkT_arr.astype(bf),
            "wvT": wvT_arr.astype(bf),
            "bkv": bkv_arr.astype(f32),
            "woT": woT_arr.astype(bf),
            "sel": sel_t.astype(bf),
            "w1": w1_arr.astype(bf),
            "w2": w2_arr.astype(bf),
            "b1": b1r.astype(f32),
            "b2a": b2ar.astype(f32),
            "b2last": b2lr.astype(f32),
        })

    nc = _get_nc()
    res = run_bass_kernel_spmd(nc, in_maps, core_ids=list(range(N_CORES)))
    _NC_CACHE["last_result"] = res
    out = np.empty((B, S, DIM), dtype=f32)
    for c in range(N_CORES):
        r = res.results[c]["out_t"]                    # [128, 8, 512]
        out[:, c * SSH:(c + 1) * SSH, :] = r.reshape(
            128, NCH, B, DIM).transpose(2, 1, 0, 3).reshape(B, SSH, DIM)
    return out


# revision 23
# speedup vs baseline: 1.7477x; 1.0479x over previous
"""HOPE block kernel for 8 Trainium2 NeuronCores.

Sequence-parallel sharding: core c owns timesteps [256c, 256(c+1)) of all 4
batches (1024 tokens) and computes ALL 8 heads locally.  The linear-attention
memory M = cumsum_t(mean_b v k^T) is split into a local (within-shard) masked
scan plus a cross-core prefix: each core AllGathers its per-shard memory sum
G_c (8 heads x 64x64, 64KB bf16) and folds sum_{c'<c} G_c' in with a
0/1-mask matmul.  Everything else (LN1/QKV/scan/Wo/LN2/CMS) is local.

ln scales/biases and the 1/B factor are folded into the projection weights
host-side; all weights are pre-arranged host-side so every DMA is a plain
[128, contiguous] transfer.
"""

import numpy as np
import ml_dtypes

import concourse.bass as bass
import concourse.bacc as bacc
import concourse.mybir as mybir
import concourse.tile as tile
from concourse.bass_utils import run_bass_kernel_spmd
from concourse.masks import make_identity

N_CORES = 8
B, S, DIM = 4, 2048, 512
H, D = 8, 64
HID = 4 * DIM
NLVL = 3
EPS = 1e-5
SSH = S // N_CORES       # 256 timesteps per core
TSH = B * SSH            # 1024 tokens per core
NT = TSH // 128          # 8 token tiles (tile t = chunk(t//4), batch(t%4))
NCH = SSH // 128         # 2 chunks of 128 steps
FP32 = mybir.dt.float32
BF16 = mybir.dt.bfloat16
AX = mybir.AxisListType.X
ALU = mybir.AluOpType
ACTF = mybir.ActivationFunctionType


def _ln_normalize(nc, pool, xt, out_bf, sq_scratch, eps_tile):
    """out_bf = (xt - mean(xt)) * rsqrt(var(xt)+EPS), per 128-token tile."""
    ssum = pool.tile([128, 1], FP32, tag="ln_s")
    sumsq = pool.tile([128, 1], FP32, tag="ln_q")
    nc.vector.tensor_reduce(ssum[:], xt[:], AX, ALU.add)
    nc.scalar.activation(sq_scratch[:], xt[:], ACTF.Square, accum_out=sumsq[:])
    negmu = pool.tile([128, 1], FP32, tag="ln_m")
    nc.vector.tensor_scalar_mul(negmu[:], ssum[:], -1.0 / DIM)
    e2 = pool.tile([128, 1], FP32, tag="ln_e")
    nc.vector.tensor_scalar_mul(e2[:], sumsq[:], 1.0 / DIM)
    mu2 = pool.tile([128, 1], FP32, tag="ln_2")
    nc.vector.tensor_tensor(mu2[:], negmu[:], negmu[:], ALU.mult)
    var = pool.tile([128, 1], FP32, tag="ln_v")
    nc.vector.tensor_tensor(var[:], e2[:], mu2[:], ALU.subtract)
    std = pool.tile([128, 1], FP32, tag="ln_d")
    nc.scalar.activation(std[:], var[:], ACTF.Sqrt, bias=eps_tile[:])
    rs = pool.tile([128, 1], FP32, tag="ln_r")
    nc.vector.reciprocal(rs[:], std[:])
    nc.vector.tensor_scalar(
        out=out_bf[:], in0=xt[:], scalar1=negmu[:], scalar2=rs[:],
        op0=ALU.add, op1=ALU.mult,
    )


def build_kernel():
    nc = bacc.Bacc(num_devices=N_CORES)

    x_t = nc.dram_tensor("x_t", [128, NT, DIM], FP32, kind="ExternalInput")
    qkw = nc.dram_tensor("qkw", [128, 4, H, 128], BF16, kind="ExternalInput")
    qk_b = nc.dram_tensor("qk_b", [128, H], FP32, kind="ExternalInput")
    wkT = nc.dram_tensor("wkT", [128, 4, DIM], BF16, kind="ExternalInput")
    wvT = nc.dram_tensor("wvT", [128, 4, DIM], BF16, kind="ExternalInput")
    bkv = nc.dram_tensor("bkv", [128, 2 * DIM], FP32, kind="ExternalInput")
    woT = nc.dram_tensor("woT", [128, 4, DIM], BF16, kind="ExternalInput")
    sel = nc.dram_tensor("sel", [128, 4, D], BF16, kind="ExternalInput")
    w1 = nc.dram_tensor("w1", [NLVL, 128, 4 * 16 * 128], BF16, kind="ExternalInput")
    w2 = nc.dram_tensor("w2", [NLVL, 128, 16 * 4 * 128], BF16, kind="ExternalInput")
    b1 = nc.dram_tensor("b1", [NLVL, 128, HID // 128], FP32, kind="ExternalInput")
    b2a = nc.dram_tensor("b2a", [2, 128, DIM // 128], FP32, kind="ExternalInput")
    b2last = nc.dram_tensor("b2last", [128, DIM], FP32, kind="ExternalInput")
    out_t = nc.dram_tensor("out_t", [128, NT, DIM], FP32, kind="ExternalOutput")

    with tile.TileContext(nc) as tc:
        with tc.tile_pool(name="dram", bufs=1, space="DRAM") as dram, \
             tc.tile_pool(name="const", bufs=1) as cpool, \
             tc.tile_pool(name="lns", bufs=4) as lnp, \
             tc.tile_pool(name="xp", bufs=1) as xpool:

            ag_in = dram.tile([D, DIM], BF16)
            ag_out = dram.tile([N_CORES * D, DIM], BF16, addr_space="Shared")

            # x first: everything downstream waits on it (split across
            # DMA queues)
            x_sb = xpool.tile([128, NT, DIM], FP32)
            for xh in range(4):
                nc.sync.dma_start(x_sb[:, 2 * xh:2 * xh + 2], x_t[:, 2 * xh:2 * xh + 2])

            identity = cpool.tile([128, 128], BF16)
            make_identity(nc, identity[:])
            # keep-mask tiled over 4 batch column blocks: mask[t, s%128]=1 if t<=s
            mask = cpool.tile([128, 512], FP32)
            nc.gpsimd.memset(mask[:], 1.0)
            for bb in range(4):
                nc.gpsimd.affine_select(
                    out=mask[:, bb * 128:(bb + 1) * 128],
                    in_=mask[:, bb * 128:(bb + 1) * 128],
                    compare_op=ALU.is_ge, fill=0.0,
                    base=0, pattern=[[1, 128]], channel_multiplier=-1,
                )

            qkw_sb = cpool.tile([128, 4, H, 128], BF16)
            nc.sync.dma_start(qkw_sb[:], qkw[:])
            qkb_sb = cpool.tile([128, H], FP32)
            nc.sync.dma_start(qkb_sb[:], qk_b[:])
            wkT_sb = cpool.tile([128, 4, DIM], BF16)
            nc.sync.dma_start(wkT_sb[:], wkT[:])
            wvT_sb = cpool.tile([128, 4, DIM], BF16)
            nc.sync.dma_start(wvT_sb[:], wvT[:])
            bkv_sb = cpool.tile([128, 2 * DIM], FP32)
            nc.sync.dma_start(bkv_sb[:], bkv[:])
            woT_sb = cpool.tile([128, 4, DIM], BF16)
            nc.sync.dma_start(woT_sb[:], woT[:])
            sel_sb = cpool.tile([128, 4, D], BF16)
            nc.sync.dma_start(sel_sb[:], sel[:])
            b2l_sb = cpool.tile([128, DIM], FP32)
            nc.sync.dma_start(b2l_sb[:], b2last[:])
            eps_sb = cpool.tile([128, 1], FP32)
            nc.vector.memset(eps_sb[:], EPS)

            # long-lived attention tiles
            yT_p = tc.tile_pool(name="yTp", bufs=1)
            yTpool = yT_p.__enter__()
            yT = yTpool.tile([128, 4, TSH], BF16)
            yTc = yTpool.tile([128, 4, TSH], BF16, name="yTc")

            with tc.tile_pool(name="kv", bufs=1) as kvp, \
                 tc.tile_pool(name="qk", bufs=1) as qkp, \
                 tc.tile_pool(name="s1w", bufs=3) as s1w:
                QT = qkp.tile([D, H, TSH], BF16)
                KT = qkp.tile([D, H, TSH], BF16)
                K_td = kvp.tile([128, NT, DIM], BF16)
                V_td = kvp.tile([128, NT, DIM], BF16)
                GcA_sb = kvp.tile([D, DIM], FP32)
                GcB_sb = kvp.tile([D, DIM], FP32)
                Gc_bf = kvp.tile([D, DIM], BF16)

                hT_ctx = tc.tile_pool(name="hT", bufs=1)
                hTp = hT_ctx.__enter__()
                hT = hTp.tile([128, 4, TSH], BF16)
                # ---- stage 1: ln1 per token tile + transpose to [dim, tok] ----
                with tc.tile_pool(name="s1p", bufs=2, space="PSUM") as s1p:
                    for t in range(NT):
                        hn = s1w.tile([128, DIM], BF16, tag="hn")
                        sq = s1w.tile([128, DIM], BF16, tag="sq")
                        _ln_normalize(nc, lnp, x_sb[:, t], hn, sq, eps_sb)
                        for a in range(4):
                            ps = s1p.tile([128, 128], BF16)
                            nc.tensor.transpose(ps[:], hn[:, a * 128:(a + 1) * 128],
                                                identity[:])
                            nc.vector.tensor_copy(hT[:, a, t * 128:(t + 1) * 128], ps[:])

                # ---- stage 2a: token-side K/V projections + local memory sums ----
                with tc.tile_pool(name="s2aw", bufs=2, space="PSUM") as s2ap, \
                     tc.tile_pool(name="s2g", bufs=1, space="PSUM") as s2gp:
                    pgA = s2gp.tile([D, DIM], FP32)
                    pgB = s2gp.tile([D, DIM], FP32)
                    for t in range(NT):
                        tcol = slice(t * 128, (t + 1) * 128)
                        psK = s2ap.tile([128, DIM], FP32, tag="psK")
                        psV = s2ap.tile([128, DIM], FP32, tag="psV")
                        for a in range(4):
                            nc.tensor.matmul(psK[:], hT[:, a, tcol], wkT_sb[:, a],
                                             start=(a == 0), stop=(a == 3))
                        for a in range(4):
                            nc.tensor.matmul(psV[:], hT[:, a, tcol], wvT_sb[:, a],
                                             start=(a == 0), stop=(a == 3))
                        nc.vector.tensor_tensor(K_td[:, t], psK[:], bkv_sb[:, 0:DIM],
                                                ALU.add)
                        nc.vector.tensor_tensor(V_td[:, t], psV[:], bkv_sb[:, DIM:],
                                                ALU.add)
                    # local memory sums; each accumulation group must be
                    # contiguous matmul instructions
                    for pg, t0 in ((pgA, 0), (pgB, 4)):
                        for h in range(H):
                            hc = slice(h * D, (h + 1) * D)
                            for tt in range(4):
                                nc.tensor.matmul(pg[:, hc], K_td[:, t0 + tt, hc],
                                                 V_td[:, t0 + tt, hc],
                                                 start=(tt == 0), stop=(tt == 3))
                    nc.vector.tensor_copy(GcA_sb[:], pgA[:])
                    nc.vector.tensor_copy(GcB_sb[:], pgB[:])
                    nc.vector.tensor_tensor(Gc_bf[:], GcA_sb[:], GcB_sb[:], ALU.add)
                    nc.sync.dma_start(ag_in[:], Gc_bf[:])

                nc.gpsimd.collective_compute(
                    "AllGather", ALU.bypass,
                    replica_groups=[list(range(N_CORES))],
                    ins=[ag_in.opt()], outs=[ag_out.opt()],
                )

                # agg DMA fires as soon as the collective lands
                agg_sb = kvp.tile([128, 4, DIM], BF16, name="agg_sb")
                nc.sync.dma_start(
                    agg_sb[:], ag_out[:].rearrange("(a p) m -> p a m", p=128))

                # ---- stage 2b: Q/K head-block projections, software-pipelined
                # with the causal T = mask o (K^T Q) matmuls AND the local
                # (prefix-free) readout y_local = sum_bp V_bp Tm_bp, so the PE
                # runs one continuous stream while the collective flies.  The
                # cross-core prefix term G @ Q is added later, folded into the
                # Wo matmul's accumulation group via yTc. ----
                tm_ctx = tc.tile_pool(name="tm", bufs=64)
                tmp_pool = tm_ctx.__enter__()
                tms = {}
                with tc.tile_pool(name="s2bp", bufs=3, space="PSUM") as s2bp, \
                     tc.tile_pool(name="pt3", bufs=3, space="PSUM") as pt3, \
                     tc.tile_pool(name="py3", bufs=2, space="PSUM") as py3:
                    def emit_qk(j):
                        for nh in range(2):
                            ncol = slice(nh * 512, (nh + 1) * 512)
                            pqk = s2bp.tile([128, 512], FP32)
                            for a in range(4):
                                nc.tensor.matmul(pqk[:], qkw_sb[:, a, j],
                                                 hT[:, a, ncol],
                                                 start=(a == 0), stop=(a == 3))
                            nc.scalar.activation(QT[:, j, ncol], pqk[0:D, :],
                                                 ACTF.Identity,
                                                 bias=qkb_sb[0:D, j:j + 1])
                            nc.scalar.activation(KT[:, j, ncol], pqk[D:128, :],
                                                 ACTF.Identity,
                                                 bias=qkb_sb[D:128, j:j + 1])

                    def emit_T(j):
                        for sc in range(NCH):
                            qcol = slice(sc * 512, (sc + 1) * 512)
                            for bp in range(B):
                                kcol = slice((sc * 4 + bp) * 128,
                                             (sc * 4 + bp) * 128 + 128)
                                pt = pt3.tile([128, 512], FP32)
                                nc.tensor.matmul(pt[:], KT[:, j, kcol],
                                                 QT[:, j, qcol])
                                tm = tmp_pool.tile([128, 512], BF16)
                                nc.vector.tensor_tensor(tm[:], pt[:], mask[:],
                                                        ALU.mult)
                                tms[(sc, j, bp)] = tm

                    def emit_ylocal(h):
                        for sc in range(NCH):
                            qcol = slice(sc * 512, (sc + 1) * 512)
                            py = py3.tile([D, 512], FP32)
                            hc = slice(h * D, (h + 1) * D)
                            for bp in range(B):
                                nc.tensor.matmul(py[:], V_td[:, sc * 4 + bp, hc],
                                                 tms[(sc, h, bp)][:],
                                                 start=(bp == 0), stop=(bp == B - 1))
                            nc.scalar.activation(
                                yT[(h % 2) * D:(h % 2) * D + D, h // 2, qcol],
                                py[:], ACTF.Copy)

                    for j in range(H + 2):
                        if j < H:
                            emit_qk(j)
                        if 1 <= j <= H:
                            emit_T(j - 1)
                        if j >= 2:
                            emit_ylocal(j - 2)
                tm_ctx.__exit__(None, None, None)
                hT_ctx.__exit__(None, None, None)

                # ---- stage 3: cross-core prefix fold -> yTc ----
                with tc.tile_pool(name="gt", bufs=1) as gtp, \
                     tc.tile_pool(name="pyc", bufs=2, space="PSUM") as pycp, \
                     tc.tile_pool(name="pgp", bufs=1, space="PSUM") as pgpp:
                    pgp = pgpp.tile([D, DIM], FP32)
                    for a in range(4):
                        nc.tensor.matmul(pgp[:], sel_sb[:, a], agg_sb[:, a],
                                         start=(a == 0), stop=(a == 3))
                    G0_bf = gtp.tile([D, DIM], BF16)
                    G1_bf = gtp.tile([D, DIM], BF16)
                    nc.vector.tensor_copy(G0_bf[:], pgp[:])
                    nc.vector.tensor_tensor(G1_bf[:], pgp[:], GcA_sb[:], ALU.add)
                    for sc in range(NCH):
                        qcol = slice(sc * 512, (sc + 1) * 512)
                        Gsc = G0_bf if sc == 0 else G1_bf
                        for h in range(H):
                            hc = slice(h * D, (h + 1) * D)
                            pyc = pycp.tile([D, 512], FP32)
                            nc.tensor.matmul(pyc[:], Gsc[:, hc], QT[:, h, qcol])
                            nc.scalar.activation(
                                yTc[(h % 2) * D:(h % 2) * D + D, h // 2, qcol],
                                pyc[:], ACTF.Copy)

            # CMS weight pool + level-0 prefetch (SBUF freed by attention)
            wts_ctx = tc.tile_pool(name="cmsw", bufs=2)
            wts = wts_ctx.__enter__()
            w1_sb0 = wts.tile([128, 4, 16, 128], BF16, tag="w1")
            nc.sync.dma_start(w1_sb0[:], w1[0].rearrange("p (a h q) -> p a h q", a=4, h=16))
            w2_sb0 = wts.tile([128, 16, 4, 128], BF16, tag="w2")
            nc.sync.dma_start(w2_sb0[:], w2[0].rearrange("p (h a q) -> p h a q", h=16, a=4))

            # ---- stage 4: Wo + residual + ln2 + transpose ----
            h2nT_p = tc.tile_pool(name="h2nT", bufs=1)
            h2_p = tc.tile_pool(name="h2", bufs=1)
            h2nT = h2nT_p.__enter__().tile([128, 4, TSH], BF16)
            h2_sb = h2_p.__enter__().tile([128, NT, DIM], FP32)
            with tc.tile_pool(name="s4w", bufs=4) as s4w, \
                 tc.tile_pool(name="s4p", bufs=2, space="PSUM") as s4p, \
                 tc.tile_pool(name="s4pt", bufs=2, space="PSUM") as s4pt:
                for t in range(NT):
                    tcol = slice(t * 128, (t + 1) * 128)
                    po = s4p.tile([128, DIM], FP32)
                    for a in range(4):
                        nc.tensor.matmul(po[:], yT[:, a, tcol], woT_sb[:, a],
                                         start=(a == 0), stop=False)
                    for a in range(4):
                        nc.tensor.matmul(po[:], yTc[:, a, tcol], woT_sb[:, a],
                                         start=False, stop=(a == 3))
                    nc.vector.tensor_tensor(h2_sb[:, t], po[:], x_sb[:, t], ALU.add)
                    hn = s4w.tile([128, DIM], BF16, tag="hn2")
                    sq = s4w.tile([128, DIM], BF16, tag="sq2")
                    _ln_normalize(nc, lnp, h2_sb[:, t], hn, sq, eps_sb)
                    for a in range(4):
                        ps = s4pt.tile([128, 128], BF16)
                        nc.tensor.transpose(ps[:], hn[:, a * 128:(a + 1) * 128], identity[:])
                        nc.vector.tensor_copy(h2nT[:, a, t * 128:(t + 1) * 128], ps[:])

            # ---- stage 5: CMS chain ----
            with tc.tile_pool(name="g", bufs=1) as gp, \
                 tc.tile_pool(name="bts", bufs=2) as bts, \
                 tc.tile_pool(name="s5o", bufs=2) as s5o, \
                 tc.tile_pool(name="s5p", bufs=4, space="PSUM") as s5p:
                g_sb = gp.tile([128, 16, TSH], BF16)
                cur = h2nT
                for lvl in range(NLVL):
                    if lvl == 0:
                        w1_sb = w1_sb0
                    else:
                        w1_sb = wts.tile([128, 4, 16, 128], BF16, tag="w1")
                        nc.sync.dma_start(
                            w1_sb[:],
                            w1[lvl].rearrange("p (a h q) -> p a h q", a=4, h=16))
                    b1_sb = bts.tile([128, HID // 128], FP32, tag="b1")
                    nc.sync.dma_start(b1_sb[:], b1[lvl])
                    for ht in range(16):
                        for nh in range(2):
                            colw = slice(nh * 512, nh * 512 + 512)
                            ps = s5p.tile([128, 512], FP32)
                            for a in range(4):
                                nc.tensor.matmul(ps[:], w1_sb[:, a, ht],
                                                 cur[:, a, colw],
                                                 start=(a == 0), stop=(a == 3))
                            nc.scalar.activation(
                                g_sb[:, ht, colw], ps[:], ACTF.Gelu_apprx_tanh,
                                bias=b1_sb[:, ht:ht + 1])
                    if lvl == 0:
                        w2_sb = w2_sb0
                    else:
                        w2_sb = wts.tile([128, 16, 4, 128], BF16, tag="w2")
                        nc.sync.dma_start(
                            w2_sb[:],
                            w2[lvl].rearrange("p (h a q) -> p h a q", h=16, a=4))
                    if lvl < 2:
                        b2_sb = bts.tile([128, 4], FP32, tag="b2")
                        nc.sync.dma_start(b2_sb[:], b2a[lvl])
                        nxt = s5o.tile([128, 4, TSH], BF16, tag="nxt")
                        for a in range(4):
                            for nh in range(2):
                                colw = slice(nh * 512, nh * 512 + 512)
                                ps = s5p.tile([128, 512], FP32)
                                for ht in range(16):
                                    nc.tensor.matmul(ps[:], w2_sb[:, ht, a],
                                                     g_sb[:, ht, colw],
                                                     start=(ht == 0), stop=(ht == 15))
                                nc.scalar.activation(
                                    nxt[:, a, colw], ps[:], ACTF.Identity,
                                    bias=b2_sb[:, a:a + 1])
                        cur = nxt
                    else:
                        # last level emits [tok, dim]; add b2 + residual, write out
                        w2r = w2_sb[:].rearrange("p h a q -> p h (a q)")
                        for t in range(NT):
                            ps = s5p.tile([128, 512], FP32)
                            for ht in range(16):
                                nc.tensor.matmul(
                                    ps[:], g_sb[:, ht, t * 128:(t + 1) * 128],
                                    w2r[:, ht],
                                    start=(ht == 0), stop=(ht == 15))
                            tmp = s5o.tile([128, DIM], FP32, tag="fin")
                            nc.vector.tensor_tensor(tmp[:], ps[:], b2l_sb[:], ALU.add)
                            nc.vector.tensor_tensor(tmp[:], tmp[:], h2_sb[:, t], ALU.add)
                            nc.sync.dma_start(out_t[:, t], tmp[:])
            h2_p.__exit__(None, None, None)
            h2nT_p.__exit__(None, None, None)
            wts_ctx.__exit__(None, None, None)
            yT_p.__exit__(None, None, None)

    nc.finalize()
    return nc


_NC_CACHE = {}


def _get_nc():
    if "nc" not in _NC_CACHE:
        _NC_CACHE["nc"] = build_kernel()
    return _NC_CACHE["nc"]


def kernel(x, Wq, Wk, Wv, Wo, ln1_w, ln1_b, ln2_w, ln2_b,
           cms_W1, cms_b1, cms_W2, cms_b2):
    bf = ml_dtypes.bfloat16
    f32 = np.float32
    x = np.asarray(x, f32)
    ln1_w = np.asarray(ln1_w, f32); ln1_b = np.asarray(ln1_b, f32)
    ln2_w = np.asarray(ln2_w, f32); ln2_b = np.asarray(ln2_b, f32)

    Wq = np.asarray(Wq, f32); Wk = np.asarray(Wk, f32); Wv = np.asarray(Wv, f32)
    Wo = np.asarray(Wo, f32)
    Wqs = Wq * ln1_w[None, :]; Wks = Wk * ln1_w[None, :]
    Wvs = (Wv * ln1_w[None, :]) / B
    bq = Wq @ ln1_b; bk = Wk @ ln1_b; bv = (Wv @ ln1_b) / B

    W1 = np.asarray(cms_W1, f32).copy(); b1v = np.asarray(cms_b1, f32).copy()
    W2 = np.asarray(cms_W2, f32); b2v = np.asarray(cms_b2, f32)
    b1v[0] = b1v[0] + ln2_b @ W1[0]
    W1[0] = W1[0] * ln2_w[:, None]

    # [128, 4a, 8j, 128m]: column block j = head j's (q|k) output dims
    QK = np.concatenate([Wqs.reshape(H, D, DIM), Wks.reshape(H, D, DIM)], axis=1)
    qkw_arr = QK.transpose(2, 0, 1).reshape(4, 128, H, 128).transpose(1, 0, 2, 3)
    qkb_arr = np.concatenate([bq.reshape(H, D), bk.reshape(H, D)], axis=1).T
    wkT_arr = Wks.T.reshape(4, 128, DIM).transpose(1, 0, 2)
    wvT_arr = Wvs.T.reshape(4, 128, DIM).transpose(1, 0, 2)
    bkv_arr = np.broadcast_to(np.concatenate([bk, bv]), (128, 2 * DIM))
    woT_arr = Wo.T.reshape(4, 128, DIM).transpose(1, 0, 2)

    w1_arr = W1.reshape(NLVL, 4, 128, 16, 128).transpose(0, 2, 1, 3, 4).reshape(
        NLVL, 128, 4 * 16 * 128)
    w2_arr = W2.reshape(NLVL, 16, 128, 4, 128).transpose(0, 2, 1, 3, 4).reshape(
        NLVL, 128, 16 * 4 * 128)
    b1r = np.ascontiguousarray(b1v.reshape(NLVL, HID // 128, 128).transpose(0, 2, 1))
    b2ar = np.ascontiguousarray(b2v[:2].reshape(2, DIM // 128, 128).transpose(0, 2, 1))
    b2lr = np.broadcast_to(b2v[2], (128, DIM)).copy()

    rows = np.arange(512)
    in_maps = []
    for c in range(N_CORES):
        xs = x[:, c * SSH:(c + 1) * SSH, :]            # [4, 256, 512]
        x_tiled = xs.reshape(B, NCH, 128, DIM).transpose(2, 1, 0, 3).reshape(
            128, NT, DIM)
        sel_arr = ((rows // D < c)[:, None] &
                   ((rows % D)[:, None] == np.arange(D)[None, :])).astype(f32)
        sel_t = sel_arr.reshape(4, 128, D).transpose(1, 0, 2)
        in_maps.append({
            "x_t": np.ascontiguousarray(x_tiled),
            "qkw": qkw_arr.astype(bf),
            "qk_b": qkb_arr.astype(f32),
            "wkT": wkT_arr.astype(bf),
            "wvT": wvT_arr.astype(bf),
            "bkv": bkv_arr.astype(f32),
            "woT": woT_arr.astype(bf),
            "sel": sel_t.astype(bf),
            "w1": w1_arr.astype(bf),
            "w2": w2_arr.astype(bf),
            "b1": b1r.astype(f32),
            "b2a": b2ar.astype(f32),
            "b2last": b2lr.astype(f32),
        })

    nc = _get_nc()
    res = run_bass_kernel_spmd(nc, in_maps, core_ids=list(range(N_CORES)))
    _NC_CACHE["last_result"] = res
    out = np.empty((B, S, DIM), dtype=f32)
    for c in range(N_CORES):
        r = res.results[c]["out_t"]                    # [128, 8, 512]
        out[:, c * SSH:(c + 1) * SSH, :] = r.reshape(
            128, NCH, B, DIM).transpose(2, 1, 0, 3).reshape(B, SSH, DIM)
    return out
